# revision 1
# baseline (speedup 1.0000x reference)
"""Trainium2 Bass kernel for the GRU caption model.

Computes: h0 = feat @ W_hp.T + b_hp; 200-step GRU with constant hidden-proj
gate pre-activations; logits = outs @ W_out.T + b_out -> [B, V, T].

Strategy: every core runs the (tiny, latency-bound) GRU redundantly; the
vocab dimension of W_out is sharded 8 ways; each core emits its own
[B, 3840, T] logits slice which the host concatenates.

All on-chip compute uses a transposed [feature-on-partitions, batch-free]
layout so the recurrent state feeds the next step's matmul directly.
"""

import numpy as np
import ml_dtypes

import concourse.bass as bass
import concourse.mybir as mybir
import concourse.tile as tile
from concourse import bacc
from concourse.bass_utils import run_bass_kernel_spmd

F32 = mybir.dt.float32
F32R = mybir.dt.float32r
BF16 = mybir.dt.bfloat16
AF = mybir.ActivationFunctionType
ALU = mybir.AluOpType

VOCAB = 30522
HID = 512
FEAT = 2048
STEPS = 200
BATCH = 32
SOS = 101
NCORES = 8
P = 128
KO = HID // P          # 4 h-chunks
GM = 3 * HID // P      # 12 gate row-groups (r: 0-3, z: 4-7, n: 8-11)
KF = FEAT // P         # 16 feat chunks
VPAD = 3840            # per-core padded vocab rows = 30 * 128
MT = VPAD // P         # 30 vocab tiles per core
TBLOCKS = [(0, 64), (64, 128), (128, 200)]  # proj t-blocks

LAST_RESULTS = None  # test harness introspection
EMIT_GRU = True    # variant switch (sim experiments)
EMIT_PROJ = True   # variant switch (sim experiments)
PROJ_MODE = 2      # 0 = matmuls only, 1 = +copies, 2 = +DMA (sim experiments)


def _r(ap):
    """Reinterpret an fp32 AP as float32r for full-rate PE streaming."""
    return ap.bitcast(F32R)


def build():
    nc = bacc.Bacc("TRN2", target_bir_lowering=False, debug=False)

    featT = nc.dram_tensor("featT", [FEAT, BATCH], F32, kind="ExternalInput")
    WhpT = nc.dram_tensor("WhpT", [FEAT, HID], F32, kind="ExternalInput")
    WihT = nc.dram_tensor("WihT", [HID, 3 * HID], BF16, kind="ExternalInput")
    WhhT = nc.dram_tensor("WhhT", [HID, 3 * HID], F32, kind="ExternalInput")
    b_ih = nc.dram_tensor("b_ih", [3 * HID], F32, kind="ExternalInput")
    b_hh = nc.dram_tensor("b_hh", [3 * HID], F32, kind="ExternalInput")
    b_hp = nc.dram_tensor("b_hp", [HID], F32, kind="ExternalInput")
    x0T = nc.dram_tensor("x0T", [HID, BATCH], BF16, kind="ExternalInput")
    WoutT = nc.dram_tensor("WoutT", [HID, VPAD], F32R, kind="ExternalInput")
    b_out = nc.dram_tensor("b_out", [VPAD], F32, kind="ExternalInput")
    OUT = nc.dram_tensor("OUT", [BATCH, VPAD, STEPS], F32, kind="ExternalOutput")

    with tile.TileContext(nc) as tc:
        with (
            tc.tile_pool(name="const", bufs=1) as const,
            tc.tile_pool(name="stream", bufs=3) as stream,
            tc.tile_pool(name="step", bufs=4) as sp,
            tc.tile_pool(name="hb", bufs=4) as hb,
            tc.tile_pool(name="outp", bufs=6) as outp,
            tc.tile_pool(name="psg", bufs=3, space="PSUM") as psg,
            tc.tile_pool(name="psp", bufs=4, space="PSUM") as psp,
        ):
            # ---- constants into SBUF ----
            wih = const.tile([P, KO, GM, P], BF16, tag="wih")
            nc.sync.dma_start(
                wih[:], WihT.rearrange("(k p) (m c) -> p k m c", p=P, c=P)
            )
            featT_sb = const.tile([P, KF, BATCH], F32, tag="featsb")
            nc.sync.dma_start(featT_sb[:], featT.rearrange("(k p) b -> p k b", p=P))
            bih_sb = const.tile([P, GM], F32, tag="bih")
            nc.sync.dma_start(bih_sb[:], b_ih.rearrange("(m p) -> p m", p=P))
            bhh_sb = const.tile([P, GM], F32, tag="bhh")
            nc.sync.dma_start(bhh_sb[:], b_hh.rearrange("(m p) -> p m", p=P))
            bhp_sb = const.tile([P, KO], F32, tag="bhp")
            nc.sync.dma_start(bhp_sb[:], b_hp.rearrange("(m p) -> p m", p=P))
            bout_sb = const.tile([P, MT], F32, tag="bout")
            nc.sync.dma_start(bout_sb[:], b_out.rearrange("(m p) -> p m", p=P))

            WhpT_r = WhpT.rearrange("(k p) h -> p k h", p=P)
            WhhT_r = WhhT.rearrange("(k p) g -> p k g", p=P)
            WoutT_r = WoutT.rearrange("(k p) v -> p k v", p=P)

            # ---- h0 = feat @ W_hp.T + b_hp (fp32, exact) ----
            ps_h = psg.tile([P, GM, BATCH], F32, tag="gates")
            for ko in range(KO):
                for kf in range(KF):
                    wt = stream.tile([P, P], F32, tag="whp")
                    nc.sync.dma_start(wt[:], WhpT_r[:, kf, ko * P:(ko + 1) * P])
                    nc.tensor.matmul(
                        ps_h[:, ko, :], wt[:], featT_sb[:, kf, :],
                        start=(kf == 0), stop=(kf == KF - 1),
                    )
            h0T = const.tile([P, KO, BATCH], F32, tag="h0T")
            for ko in range(KO):
                nc.scalar.activation(
                    h0T[:, ko, :], ps_h[:, ko, :], AF.Identity,
                    bias=bhp_sb[:, ko, None], scale=1.0,
                )
            h0_half = const.tile([P, KO, BATCH], F32, tag="h0h")
            nc.scalar.mul(h0_half[:], h0T[:], 0.5)

            # ---- gh = h0 @ W_hh.T + b_hh (fp32, exact; step-invariant) ----
            ps_g = psg.tile([P, GM, BATCH], F32, tag="gates")
            for m in range(GM):
                for k in range(KO):
                    wt = stream.tile([P, P], F32, tag="whh")
                    nc.sync.dma_start(wt[:], WhhT_r[:, k, m * P:(m + 1) * P])
                    nc.tensor.matmul(
                        ps_g[:, m, :], wt[:], h0T[:, k, :],
                        start=(k == 0), stop=(k == KO - 1),
                    )
            ghT = const.tile([P, GM, BATCH], F32, tag="ghT")
            for m in range(GM):
                nc.scalar.activation(
                    ghT[:, m, :], ps_g[:, m, :], AF.Identity,
                    bias=bhh_sb[:, m, None], scale=1.0,
                )
            # C_rz = gh_rz + b_ih_rz ; hn2 = 0.5*gh_n ; E_n = hn2 + b_ih_n
            C_rz = const.tile([P, 8, BATCH], F32, tag="Crz")
            nc.vector.tensor_add(
                C_rz[:], ghT[:, 0:8, :],
                bih_sb[:, 0:8, None].to_broadcast((P, 8, BATCH)),
            )
            hn2 = const.tile([P, KO, BATCH], F32, tag="hn2")
            nc.scalar.mul(hn2[:], ghT[:, 8:12, :], 0.5)
            E_n = const.tile([P, KO, BATCH], F32, tag="En")
            nc.vector.tensor_add(
                E_n[:], hn2[:],
                bih_sb[:, 8:12, None].to_broadcast((P, KO, BATCH)),
            )

            # resT blocks: col = b*bsize + (t - t0), per h-chunk ko
            resT = []
            for j, (t0, t1) in enumerate(TBLOCKS):
                bs = t1 - t0
                rt = const.tile(
                    [P, KO, BATCH, bs], F32R, tag=f"resT{j}", name=f"resT{j}"
                )
                resT.append(rt)

            prev = hb.tile([P, KO, BATCH], BF16, tag="hb")
            nc.sync.dma_start(prev[:], x0T.rearrange("(k p) b -> p k b", p=P))

            def proj_block(j):
                t0, t1 = TBLOCKS[j]
                bs = t1 - t0
                gb = 4
                N = gb * bs
                for m in range(MT):
                    wt = stream.tile([P, KO, P], F32R, tag="wout")
                    nc.sync.dma_start(wt[:], WoutT_r[:, :, m * P:(m + 1) * P])
                    for g in range(BATCH // gb):
                        ps_full = psp.tile([P, 288], F32, tag="pp", name="pp")
                        ps = ps_full[:, :N]
                        for k in range(KO):
                            nc.tensor.matmul(
                                ps,
                                wt[:, k, :],
                                resT[j][:, k, gb * g:gb * g + gb, :],
                                start=(k == 0), stop=(k == KO - 1),
                            )
                        if PROJ_MODE == 0:
                            continue
                        ob_full = outp.tile([P, 288], F32, tag="ob", name="ob")
                        ob = ob_full[:, :N]
                        if (m + g) % 2 == 0:
                            nc.scalar.activation(
                                ob, ps, AF.Identity,
                                bias=bout_sb[:, m, None], scale=1.0,
                            )
                        else:
                            nc.vector.tensor_scalar_add(ob, ps, bout_sb[:, m, None])
                        if PROJ_MODE >= 2:
                            dst = OUT[
                                gb * g:gb * g + gb, m * P:(m + 1) * P, t0:t1
                            ].rearrange("b v t -> v b t")
                            nc.sync.dma_start(
                                dst, ob.rearrange("p (b t) -> p b t", b=gb)
                            )

            # ---- GRU steps ----
            if not EMIT_GRU:
                for j in range(len(TBLOCKS)):
                    nc.vector.memset(resT[j][:], 0.25)
                    proj_block(j)
            mm_order = [8, 9, 10, 11] + list(range(8))  # n-gates first
            for t in range(STEPS if EMIT_GRU else 0):
                ps = psg.tile([P, GM, BATCH], F32, tag="gates")
                for m in mm_order:
                    for k in range(KO):
                        nc.tensor.matmul(
                            ps[:, m, :], wih[:, k, m, :], prev[:, k, :],
                            start=(k == 0), stop=(k == KO - 1),
                        )
                s_rz = sp.tile([P, 8, BATCH], F32, tag="srz")
                nc.vector.tensor_add(s_rz[:], ps[:, 0:8, :], C_rz[:])
                t_rz = sp.tile([P, 8, BATCH], F32, tag="trz")
                nc.scalar.activation(t_rz[:], s_rz[:], AF.Tanh, scale=0.5)
                a = sp.tile([P, KO, BATCH], F32, tag="a")
                nc.vector.tensor_mul(a[:], t_rz[:, 0:4, :], hn2[:])
                sn1 = sp.tile([P, KO, BATCH], F32, tag="sn1")
                nc.vector.tensor_add(sn1[:], ps[:, 8:12, :], E_n[:])
                sn2 = sp.tile([P, KO, BATCH], F32, tag="sn2")
                nc.vector.tensor_add(sn2[:], sn1[:], a[:])
                n = sp.tile([P, KO, BATCH], F32, tag="n")
                nc.scalar.activation(n[:], sn2[:], AF.Tanh, scale=1.0)
                q = sp.tile([P, KO, BATCH], F32, tag="q")
                nc.vector.tensor_sub(q[:], h0T[:], n[:])
                w2 = sp.tile([P, KO, BATCH], F32, tag="w2")
                nc.vector.scalar_tensor_tensor(
                    w2[:], t_rz[:, 4:8, :], 0.5, q[:], ALU.mult, ALU.mult
                )
                p2 = sp.tile([P, KO, BATCH], F32, tag="p2")
                nc.vector.scalar_tensor_tensor(
                    p2[:], n[:], 0.5, h0_half[:], ALU.mult, ALU.add
                )
                nxt = hb.tile([P, KO, BATCH], BF16, tag="hb")
                nc.vector.tensor_add(nxt[:], w2[:], p2[:])
                j = next(i for i, (a, b) in enumerate(TBLOCKS) if a <= t < b)
                t0 = TBLOCKS[j][0]
                nc.gpsimd.tensor_add(resT[j][:, :, :, t - t0], w2[:], p2[:])
                prev = nxt
                if t == TBLOCKS[j][1] - 1 and EMIT_PROJ:
                    proj_block(j)

    nc.compile()
    return nc


def _shard_inputs(feat, W_hp, b_hp, W_ih, W_hh, b_ih, b_hh, embed, W_out, b_out):
    bf = ml_dtypes.bfloat16
    featT = np.ascontiguousarray(feat.T, dtype=np.float32)
    WhpT = np.ascontiguousarray(W_hp.T, dtype=np.float32)
    WihT = np.ascontiguousarray(W_ih.T).astype(bf)
    WhhT = np.ascontiguousarray(W_hh.T, dtype=np.float32)
    x0T = np.ascontiguousarray(
        np.repeat(np.asarray(embed)[SOS][:, None], BATCH, axis=1)
    ).astype(bf)
    Wo = np.zeros((NCORES * VPAD, HID), np.float32)
    Wo[:VOCAB] = W_out
    bo = np.zeros((NCORES * VPAD,), np.float32)
    bo[:VOCAB] = b_out
    common = dict(
        featT=featT, WhpT=WhpT, WihT=WihT, WhhT=WhhT,
        b_ih=np.asarray(b_ih, np.float32), b_hh=np.asarray(b_hh, np.float32),
        b_hp=np.asarray(b_hp, np.float32), x0T=x0T,
    )
    in_maps = []
    for c in range(NCORES):
        sl = slice(c * VPAD, (c + 1) * VPAD)
        m = dict(common)
        m["WoutT"] = np.ascontiguousarray(Wo[sl].T)
        m["b_out"] = bo[sl].copy()
        in_maps.append(m)
    return in_maps


def kernel(**inputs):
    global LAST_RESULTS
    args = {k: np.asarray(v) for k, v in inputs.items()}
    in_maps = _shard_inputs(
        args["feat"], args["W_hp"], args["b_hp"], args["W_ih"], args["W_hh"],
        args["b_ih"], args["b_hh"], args["embed"], args["W_out"], args["b_out"],
    )
    nc = build()
    res = run_bass_kernel_spmd(nc, in_maps, core_ids=list(range(NCORES)))
    LAST_RESULTS = res
    out = np.concatenate([r["OUT"] for r in res.results], axis=1)[:, :VOCAB, :]
    return np.ascontiguousarray(out, dtype=np.float32)



# revision 6
# speedup vs baseline: 1.5256x; 1.5256x over previous
"""Trainium2 Bass kernel for the GRU caption model.

Computes: h0 = feat @ W_hp.T + b_hp; 200-step GRU with constant hidden-proj
gate pre-activations; logits = outs @ W_out.T + b_out -> [B, V, T].

Strategy: every core runs the (tiny, latency-bound) GRU redundantly; the
vocab dimension of W_out is sharded 8 ways; each core emits its own
[B, 3840, T] logits slice which the host concatenates.

Schedule: GRU steps are grouped into 8-step chunks. While the GRU runs
chunk c+1, the vocab projection for chunk c's hidden states is interleaved
into the PE queue a few matmuls per step, keeping the tensor engine
continuously busy (max p-state) and hiding the recurrence's cross-engine
dependency-chain latency. Output DMAs use 1024B contiguous runs; the host
unscrambles the [MT, 128, NCH, B, TC] layout for free.

The constant gate pre-activations C_rz = gh_rz + b_ih_rz and E_n
(= 0.5*gh_n + b_ih_n) are preloaded into PSUM each step so the gate
matmuls accumulate on top of them (start=False) and tanh reads PSUM
directly, shortening the per-step chain.
"""

import numpy as np
import ml_dtypes

import concourse.bass as bass
import concourse.mybir as mybir
import concourse.tile as tile
from concourse import bacc
from concourse.bass_utils import run_bass_kernel_spmd

F32 = mybir.dt.float32
BF16 = mybir.dt.bfloat16
AF = mybir.ActivationFunctionType
ALU = mybir.AluOpType

VOCAB = 30522
HID = 512
FEAT = 2048
STEPS = 200
BATCH = 32
SOS = 101
NCORES = 8
P = 128
KO = HID // P          # 4 h-chunks
GM = 3 * HID // P      # 12 gate row-groups (r: 0-3, z: 4-7, n: 8-11)
KF = FEAT // P         # 16 feat chunks
VPAD = 3840            # per-core padded vocab rows = 30 * 128
MT = VPAD // P         # 30 vocab tiles per core
TC = 8                 # GRU steps per projection chunk
NCH = STEPS // TC      # 25 chunks
NPROJ = BATCH * TC     # moving free size per proj matmul = 256

LAST_RESULTS = None    # test harness introspection
PRELOAD = "off"        # 'pool' | 'dve' | 'off' — PSUM preload engine
                       # (note: CoreSim rejects non-PE psum writes before a
                       # start=True matmul, so only 'off' is currently legal)
UNITS_PER_STEP = 4     # proj units interleaved per GRU step


def build():
    nc = bacc.Bacc("TRN2", target_bir_lowering=False, debug=False)

    featT = nc.dram_tensor("featT", [FEAT, BATCH], F32, kind="ExternalInput")
    WhpT = nc.dram_tensor("WhpT", [FEAT, HID], F32, kind="ExternalInput")
    WihT = nc.dram_tensor("WihT", [HID, 3 * HID], BF16, kind="ExternalInput")
    WhhT = nc.dram_tensor("WhhT", [HID, 3 * HID], F32, kind="ExternalInput")
    b_ih = nc.dram_tensor("b_ih", [3 * HID], F32, kind="ExternalInput")
    b_hh = nc.dram_tensor("b_hh", [3 * HID], F32, kind="ExternalInput")
    b_hp = nc.dram_tensor("b_hp", [HID], F32, kind="ExternalInput")
    x0T = nc.dram_tensor("x0T", [HID, BATCH], BF16, kind="ExternalInput")
    WoutT = nc.dram_tensor("WoutT", [HID, VPAD], BF16, kind="ExternalInput")
    b_out = nc.dram_tensor("b_out", [VPAD], F32, kind="ExternalInput")
    OUT = nc.dram_tensor("OUT", [MT, P, NCH, BATCH, TC], F32, kind="ExternalOutput")

    with tile.TileContext(nc) as tc:
        with (
            tc.tile_pool(name="const", bufs=1) as const,
            tc.tile_pool(name="stream", bufs=3) as stream,
            tc.tile_pool(name="step", bufs=3) as sp,
            tc.tile_pool(name="outp", bufs=6) as outp,
            tc.tile_pool(name="psg", bufs=2, space="PSUM") as psg,
            tc.tile_pool(name="psp", bufs=2, space="PSUM") as psp,
        ):
            # ---- constants into SBUF ----
            featT_sb = const.tile([P, KF, BATCH], F32, tag="featsb")
            nc.sync.dma_start(featT_sb[:], featT.rearrange("(k p) b -> p k b", p=P))
            bih_sb = const.tile([P, GM], F32, tag="bih")
            nc.sync.dma_start(bih_sb[:], b_ih.rearrange("(m p) -> p m", p=P))
            bhh_sb = const.tile([P, GM], F32, tag="bhh")
            nc.sync.dma_start(bhh_sb[:], b_hh.rearrange("(m p) -> p m", p=P))
            bhp_sb = const.tile([P, KO], F32, tag="bhp")
            nc.sync.dma_start(bhp_sb[:], b_hp.rearrange("(m p) -> p m", p=P))
            bout_sb = const.tile([P, MT], F32, tag="bout")
            nc.sync.dma_start(bout_sb[:], b_out.rearrange("(m p) -> p m", p=P))
            half_sb = const.tile([P, 1], F32, tag="half")
            nc.vector.memset(half_sb[:], 0.5)
            x0_sb = const.tile([P, KO, BATCH], BF16, tag="x0")
            nc.sync.dma_start(x0_sb[:], x0T.rearrange("(k p) b -> p k b", p=P))
            wih = const.tile([P, KO, GM, P], BF16, tag="wih")
            nc.sync.dma_start(
                wih[:], WihT.rearrange("(k p) (m c) -> p k m c", p=P, c=P)
            )
            wout = const.tile([P, KO, VPAD], BF16, tag="wout")
            nc.sync.dma_start(wout[:], WoutT.rearrange("(k p) v -> p k v", p=P))

            WhpT_r = WhpT.rearrange("(k p) h -> p k h", p=P)
            WhhT_r = WhhT.rearrange("(k p) g -> p k g", p=P)

            # ---- h0 = feat @ W_hp.T + b_hp (fp32, exact) ----
            ps_h = psg.tile([P, GM, BATCH], F32, tag="gates")
            for ko in range(KO):
                for kf in range(KF):
                    wt = stream.tile([P, P], F32, tag="whp")
                    nc.sync.dma_start(wt[:], WhpT_r[:, kf, ko * P:(ko + 1) * P])
                    nc.tensor.matmul(
                        ps_h[:, ko, :], wt[:], featT_sb[:, kf, :],
                        start=(kf == 0), stop=(kf == KF - 1),
                    )
            h0T = const.tile([P, KO, BATCH], F32, tag="h0T")
            for ko in range(KO):
                nc.scalar.activation(
                    h0T[:, ko, :], ps_h[:, ko, :], AF.Identity,
                    bias=bhp_sb[:, ko, None], scale=1.0,
                )
            h0_half = const.tile([P, KO, BATCH], F32, tag="h0h")
            nc.scalar.mul(h0_half[:], h0T[:], 0.5)

            # ---- gh = h0 @ W_hh.T + b_hh (fp32, exact; step-invariant) ----
            ps_g = psg.tile([P, GM, BATCH], F32, tag="gates")
            for m in range(GM):
                for k in range(KO):
                    wt = stream.tile([P, P], F32, tag="whh")
                    nc.sync.dma_start(wt[:], WhhT_r[:, k, m * P:(m + 1) * P])
                    nc.tensor.matmul(
                        ps_g[:, m, :], wt[:], h0T[:, k, :],
                        start=(k == 0), stop=(k == KO - 1),
                    )
            ghT = const.tile([P, GM, BATCH], F32, tag="ghT")
            for m in range(GM):
                nc.scalar.activation(
                    ghT[:, m, :], ps_g[:, m, :], AF.Identity,
                    bias=bhh_sb[:, m, None], scale=1.0,
                )
            # C_rz = gh_rz + b_ih_rz ; hn2 = 0.5*gh_n ; E_n = hn2 + b_ih_n
            C_rz = const.tile([P, 8, BATCH], F32, tag="Crz")
            nc.vector.tensor_add(
                C_rz[:], ghT[:, 0:8, :],
                bih_sb[:, 0:8, None].to_broadcast((P, 8, BATCH)),
            )
            hn2 = const.tile([P, KO, BATCH], F32, tag="hn2")
            nc.scalar.mul(hn2[:], ghT[:, 8:12, :], 0.5)
            E_n = const.tile([P, KO, BATCH], F32, tag="En")
            nc.vector.tensor_add(
                E_n[:], hn2[:],
                bih_sb[:, 8:12, None].to_broadcast((P, KO, BATCH)),
            )

            # hidden-state history, bf16, chunked by TC steps
            res = [
                const.tile([P, KO, BATCH, TC], BF16, tag=f"res{c}", name=f"res{c}")
                for c in range(NCH)
            ]

            # ---- projection unit: one vocab tile m of chunk c ----
            def proj_unit(c, m):
                ps2 = psp.tile([P, NPROJ], F32, tag="pp", name="pp")
                for k in range(KO):
                    nc.tensor.matmul(
                        ps2, wout[:, k, m * P:(m + 1) * P], res[c][:, k, :, :],
                        start=(k == 0), stop=(k == KO - 1),
                    )
                ob = outp.tile([P, NPROJ], F32, tag="ob", name="ob")
                if m % 2 == 0:
                    nc.scalar.activation(
                        ob, ps2, AF.Identity, bias=bout_sb[:, m, None], scale=1.0
                    )
                else:
                    nc.vector.tensor_scalar_add(ob, ps2, bout_sb[:, m, None])
                nc.sync.dma_start(
                    OUT[m, :, c, :, :], ob.rearrange("p (b t) -> p b t", b=BATCH)
                )

            pending = []     # (c, m) proj units ready to emit
            pend_i = 0

            def emit_pending(limit):
                nonlocal pend_i
                done = 0
                while pend_i < len(pending) and done < limit:
                    proj_unit(*pending[pend_i])
                    pend_i += 1
                    done += 1

            def preload(ps):
                if PRELOAD == "pool":
                    nc.gpsimd.tensor_copy(ps[:, 0:8, :], C_rz[:])
                    nc.gpsimd.tensor_copy(ps[:, 8:12, :], E_n[:])
                elif PRELOAD == "dve":
                    nc.vector.tensor_copy(ps[:, 0:8, :], C_rz[:])
                    nc.vector.tensor_copy(ps[:, 8:12, :], E_n[:])

            # ---- GRU steps ----
            ps_cur = psg.tile([P, GM, BATCH], F32, tag="gates")
            preload(ps_cur)
            prev = x0_sb
            for t in range(STEPS):
                c, ti = t // TC, t % TC
                use_pre = PRELOAD != "off"
                for m in range(GM):
                    for k in range(KO):
                        nc.tensor.matmul(
                            ps_cur[:, m, :], wih[:, k, m, :], prev[:, k, :],
                            start=(k == 0 and not use_pre), stop=(k == KO - 1),
                        )
                # preload the *next* step's psum early (runs during this chain)
                ps_next = psg.tile([P, GM, BATCH], F32, tag="gates")
                preload(ps_next)

                if use_pre:
                    t_r = sp.tile([P, KO, BATCH], F32, tag="tr")
                    nc.scalar.activation(t_r, ps_cur[:, 0:4, :], AF.Tanh, scale=0.5)
                    t_z = sp.tile([P, KO, BATCH], F32, tag="tz")
                    nc.scalar.activation(t_z, ps_cur[:, 4:8, :], AF.Tanh, scale=0.5)
                else:
                    s_r = sp.tile([P, KO, BATCH], F32, tag="sr")
                    nc.vector.tensor_add(s_r, ps_cur[:, 0:4, :], C_rz[:, 0:4, :])
                    t_r = sp.tile([P, KO, BATCH], F32, tag="tr")
                    nc.scalar.activation(t_r, s_r, AF.Tanh, scale=0.5)
                    s_z = sp.tile([P, KO, BATCH], F32, tag="sz")
                    nc.vector.tensor_add(s_z, ps_cur[:, 4:8, :], C_rz[:, 4:8, :])
                    t_z = sp.tile([P, KO, BATCH], F32, tag="tz")
                    nc.scalar.activation(t_z, s_z, AF.Tanh, scale=0.5)
                # a = tanh(0.5 s_r) * 0.5*gh_n  (the r-gate modulation)
                a = sp.tile([P, KO, BATCH], F32, tag="a")
                nc.gpsimd.tensor_mul(a, t_r, hn2)
                # sn2 = gi_n + E_n + a
                sn2 = sp.tile([P, KO, BATCH], F32, tag="sn2")
                if use_pre:
                    nc.vector.tensor_add(sn2, ps_cur[:, 8:12, :], a)
                else:
                    sn1 = sp.tile([P, KO, BATCH], F32, tag="sn1")
                    nc.vector.tensor_add(sn1, ps_cur[:, 8:12, :], E_n[:])
                    nc.vector.tensor_add(sn2, sn1, a)
                n = sp.tile([P, KO, BATCH], F32, tag="n")
                nc.scalar.activation(n, sn2, AF.Tanh, scale=1.0)
                # h = (0.5 - 0.5 t_z) * n + (h0/2 + 0.5 t_z * h0)
                c1 = sp.tile([P, KO, BATCH], F32, tag="c1")
                nc.scalar.activation(c1, t_z, AF.Identity, scale=-0.5, bias=half_sb[:])
                th0 = sp.tile([P, KO, BATCH], F32, tag="th0")
                nc.gpsimd.scalar_tensor_tensor(
                    th0, t_z, 0.5, h0T[:], ALU.mult, ALU.mult
                )
                c2 = sp.tile([P, KO, BATCH], F32, tag="c2")
                nc.gpsimd.tensor_add(c2, th0, h0_half[:])
                m1 = sp.tile([P, KO, BATCH], F32, tag="m1")
                nc.vector.tensor_mul(m1, c1, n)
                dst = res[c][:, :, :, ti]
                nc.vector.tensor_add(dst, m1, c2)
                prev = res[c][:, :, :, ti]
                ps_cur = ps_next

                # interleave projection work for the previous chunk
                emit_pending(UNITS_PER_STEP)
                if ti == TC - 1:
                    pending.extend((c, m) for m in range(MT))

            # drain the last chunk's projection
            emit_pending(len(pending))

    nc.compile()
    return nc


def _shard_inputs(feat, W_hp, b_hp, W_ih, W_hh, b_ih, b_hh, embed, W_out, b_out):
    bf = ml_dtypes.bfloat16
    featT = np.ascontiguousarray(feat.T, dtype=np.float32)
    WhpT = np.ascontiguousarray(W_hp.T, dtype=np.float32)
    WihT = np.ascontiguousarray(W_ih.T).astype(bf)
    WhhT = np.ascontiguousarray(W_hh.T, dtype=np.float32)
    x0T = np.ascontiguousarray(
        np.repeat(np.asarray(embed)[SOS][:, None], BATCH, axis=1)
    ).astype(bf)
    Wo = np.zeros((NCORES * VPAD, HID), np.float32)
    Wo[:VOCAB] = W_out
    bo = np.zeros((NCORES * VPAD,), np.float32)
    bo[:VOCAB] = b_out
    common = dict(
        featT=featT, WhpT=WhpT, WihT=WihT, WhhT=WhhT,
        b_ih=np.asarray(b_ih, np.float32), b_hh=np.asarray(b_hh, np.float32),
        b_hp=np.asarray(b_hp, np.float32), x0T=x0T,
    )
    in_maps = []
    for c in range(NCORES):
        sl = slice(c * VPAD, (c + 1) * VPAD)
        m = dict(common)
        m["WoutT"] = np.ascontiguousarray(Wo[sl].T).astype(bf)
        m["b_out"] = bo[sl].copy()
        in_maps.append(m)
    return in_maps


def kernel(**inputs):
    global LAST_RESULTS
    args = {k: np.asarray(v) for k, v in inputs.items()}
    in_maps = _shard_inputs(
        args["feat"], args["W_hp"], args["b_hp"], args["W_ih"], args["W_hh"],
        args["b_ih"], args["b_hh"], args["embed"], args["W_out"], args["b_out"],
    )
    nc = build()
    res = run_bass_kernel_spmd(nc, in_maps, core_ids=list(range(NCORES)))
    LAST_RESULTS = res
    # OUT per core: [MT, P, NCH, B, TC] -> [B, VPAD, T]
    parts = []
    for r in res.results:
        o = r["OUT"]  # [30, 128, 25, 32, 8]
        o = o.transpose(3, 0, 1, 2, 4).reshape(BATCH, VPAD, STEPS)
        parts.append(o)
    out = np.concatenate(parts, axis=1)[:, :VOCAB, :]
    return np.ascontiguousarray(out, dtype=np.float32)


# revision 9
# speedup vs baseline: 1.6736x; 1.0970x over previous
"""Trainium2 Bass kernel for the GRU caption model.

Computes: h0 = feat @ W_hp.T + b_hp; 200-step GRU with constant hidden-proj
gate pre-activations; logits = outs @ W_out.T + b_out -> [B, V, T].

Strategy: every core runs the (tiny, latency-bound) GRU redundantly; the
vocab dimension of W_out is sharded 8 ways; each core emits its own
[B, 3840, T] logits slice which the host concatenates.

Schedule: GRU steps are grouped into 8-step chunks. While the GRU runs
chunk c+1, the vocab projection for chunk c's hidden states is interleaved
into the PE queue a few matmuls per step, keeping the tensor engine
continuously busy (max p-state) and hiding the recurrence's cross-engine
dependency-chain latency. Output DMAs use 1024B contiguous runs; the host
unscrambles the [MT, 128, NCH, B, TC] layout for free.

The constant gate pre-activations C_rz = gh_rz + b_ih_rz and E_n
(= 0.5*gh_n + b_ih_n) are preloaded into PSUM each step so the gate
matmuls accumulate on top of them (start=False) and tanh reads PSUM
directly, shortening the per-step chain.
"""

import numpy as np
import ml_dtypes

import concourse.bass as bass
import concourse.mybir as mybir
import concourse.tile as tile
from concourse import bacc
from concourse.bass_utils import run_bass_kernel_spmd

F32 = mybir.dt.float32
BF16 = mybir.dt.bfloat16
AF = mybir.ActivationFunctionType
ALU = mybir.AluOpType

VOCAB = 30522
HID = 512
FEAT = 2048
STEPS = 200
BATCH = 32
SOS = 101
NCORES = 8
P = 128
KO = HID // P          # 4 h-chunks
GM = 3 * HID // P      # 12 gate row-groups (r: 0-3, z: 4-7, n: 8-11)
KF = FEAT // P         # 16 feat chunks
VPAD = 3840            # per-core padded vocab rows = 30 * 128
MT = VPAD // P         # 30 vocab tiles per core
TC = 8                 # GRU steps per projection chunk
NCH = STEPS // TC      # 25 chunks
NPROJ = BATCH * TC     # moving free size per proj matmul = 256

LAST_RESULTS = None    # test harness introspection
PRELOAD = "off"        # 'pool' | 'dve' | 'off' — PSUM preload engine
                       # (note: CoreSim rejects non-PE psum writes before a
                       # start=True matmul, so only 'off' is currently legal)
UNITS_PER_STEP = 4     # proj units interleaved per GRU step


def build():
    nc = bacc.Bacc("TRN2", target_bir_lowering=False, debug=False)

    featT = nc.dram_tensor("featT", [FEAT, BATCH], F32, kind="ExternalInput")
    WhpT = nc.dram_tensor("WhpT", [FEAT, HID], F32, kind="ExternalInput")
    WihT = nc.dram_tensor("WihT", [HID, 3 * HID], BF16, kind="ExternalInput")
    WhhT = nc.dram_tensor("WhhT", [HID, 3 * HID], F32, kind="ExternalInput")
    b_ih = nc.dram_tensor("b_ih", [3 * HID], F32, kind="ExternalInput")
    b_hh = nc.dram_tensor("b_hh", [3 * HID], F32, kind="ExternalInput")
    b_hp = nc.dram_tensor("b_hp", [HID], F32, kind="ExternalInput")
    x0T = nc.dram_tensor("x0T", [HID, BATCH], BF16, kind="ExternalInput")
    WoutT = nc.dram_tensor("WoutT", [HID, VPAD], BF16, kind="ExternalInput")
    b_out = nc.dram_tensor("b_out", [VPAD], F32, kind="ExternalInput")
    OUT = nc.dram_tensor("OUT", [MT, P, NCH, BATCH, TC], F32, kind="ExternalOutput")

    with tile.TileContext(nc) as tc:
        with (
            tc.tile_pool(name="const", bufs=1) as const,
            tc.tile_pool(name="stream", bufs=3) as stream,
            tc.tile_pool(name="step", bufs=3) as sp,
            tc.tile_pool(name="outp", bufs=6) as outp,
            tc.tile_pool(name="psg", bufs=2, space="PSUM") as psg,
            tc.tile_pool(name="psp", bufs=2, space="PSUM") as psp,
        ):
            # ---- constants into SBUF ----
            featT_sb = const.tile([P, KF, BATCH], F32, tag="featsb")
            nc.sync.dma_start(featT_sb[:], featT.rearrange("(k p) b -> p k b", p=P))
            bih_sb = const.tile([P, GM], F32, tag="bih")
            nc.sync.dma_start(bih_sb[:], b_ih.rearrange("(m p) -> p m", p=P))
            bhh_sb = const.tile([P, GM], F32, tag="bhh")
            nc.sync.dma_start(bhh_sb[:], b_hh.rearrange("(m p) -> p m", p=P))
            bhp_sb = const.tile([P, KO], F32, tag="bhp")
            nc.sync.dma_start(bhp_sb[:], b_hp.rearrange("(m p) -> p m", p=P))
            bout_sb = const.tile([P, MT], F32, tag="bout")
            nc.sync.dma_start(bout_sb[:], b_out.rearrange("(m p) -> p m", p=P))
            halves = const.tile([P, KO, BATCH], F32, tag="halves")
            nc.vector.memset(halves[:], 0.5)
            x0_sb = const.tile([P, KO, BATCH], BF16, tag="x0")
            nc.sync.dma_start(x0_sb[:], x0T.rearrange("(k p) b -> p k b", p=P))
            wih = const.tile([P, KO, GM, P], BF16, tag="wih")
            nc.sync.dma_start(
                wih[:], WihT.rearrange("(k p) (m c) -> p k m c", p=P, c=P)
            )
            wout = const.tile([P, KO, VPAD], BF16, tag="wout")
            nc.sync.dma_start(wout[:], WoutT.rearrange("(k p) v -> p k v", p=P))

            WhpT_r = WhpT.rearrange("(k p) h -> p k h", p=P)
            WhhT_r = WhhT.rearrange("(k p) g -> p k g", p=P)

            # ---- h0 = feat @ W_hp.T + b_hp (fp32, exact) ----
            ps_h = psg.tile([P, GM, BATCH], F32, tag="gates")
            for ko in range(KO):
                for kf in range(KF):
                    wt = stream.tile([P, P], F32, tag="whp")
                    nc.sync.dma_start(wt[:], WhpT_r[:, kf, ko * P:(ko + 1) * P])
                    nc.tensor.matmul(
                        ps_h[:, ko, :], wt[:], featT_sb[:, kf, :],
                        start=(kf == 0), stop=(kf == KF - 1),
                    )
            h0T = const.tile([P, KO, BATCH], F32, tag="h0T")
            for ko in range(KO):
                nc.scalar.activation(
                    h0T[:, ko, :], ps_h[:, ko, :], AF.Identity,
                    bias=bhp_sb[:, ko, None], scale=1.0,
                )
            h0_half = const.tile([P, KO, BATCH], F32, tag="h0h")
            nc.scalar.mul(h0_half[:], h0T[:], 0.5)

            # ---- gh = h0 @ W_hh.T + b_hh (fp32, exact; step-invariant) ----
            ps_g = psg.tile([P, GM, BATCH], F32, tag="gates")
            for m in range(GM):
                for k in range(KO):
                    wt = stream.tile([P, P], F32, tag="whh")
                    nc.sync.dma_start(wt[:], WhhT_r[:, k, m * P:(m + 1) * P])
                    nc.tensor.matmul(
                        ps_g[:, m, :], wt[:], h0T[:, k, :],
                        start=(k == 0), stop=(k == KO - 1),
                    )
            ghT = const.tile([P, GM, BATCH], F32, tag="ghT")
            for m in range(GM):
                nc.scalar.activation(
                    ghT[:, m, :], ps_g[:, m, :], AF.Identity,
                    bias=bhh_sb[:, m, None], scale=1.0,
                )
            # C_rz = gh_rz + b_ih_rz ; hn2 = 0.5*gh_n ; E_n = hn2 + b_ih_n
            C_rz = const.tile([P, 8, BATCH], F32, tag="Crz")
            nc.vector.tensor_add(
                C_rz[:], ghT[:, 0:8, :],
                bih_sb[:, 0:8, None].to_broadcast((P, 8, BATCH)),
            )
            hn2 = const.tile([P, KO, BATCH], F32, tag="hn2")
            nc.scalar.mul(hn2[:], ghT[:, 8:12, :], 0.5)
            E_n = const.tile([P, KO, BATCH], F32, tag="En")
            nc.vector.tensor_add(
                E_n[:], hn2[:],
                bih_sb[:, 8:12, None].to_broadcast((P, KO, BATCH)),
            )

            # hidden-state history, bf16, chunked by TC steps
            res = [
                const.tile([P, KO, BATCH, TC], BF16, tag=f"res{c}", name=f"res{c}")
                for c in range(NCH)
            ]

            # ---- projection unit: one vocab tile m of chunk c ----
            def proj_unit(c, m):
                ps2 = psp.tile([P, NPROJ], F32, tag="pp", name="pp")
                for k in range(KO):
                    nc.tensor.matmul(
                        ps2, wout[:, k, m * P:(m + 1) * P], res[c][:, k, :, :],
                        start=(k == 0), stop=(k == KO - 1),
                    )
                ob = outp.tile([P, NPROJ], F32, tag="ob", name="ob")
                nc.gpsimd.tensor_scalar_add(ob, ps2, bout_sb[:, m, None])
                nc.sync.dma_start(
                    OUT[m, :, c, :, :], ob.rearrange("p (b t) -> p b t", b=BATCH)
                )

            pending = []     # (c, m) proj units ready to emit
            pend_i = 0

            def emit_pending(limit):
                nonlocal pend_i
                done = 0
                while pend_i < len(pending) and done < limit:
                    proj_unit(*pending[pend_i])
                    pend_i += 1
                    done += 1

            def preload(ps):
                if PRELOAD == "pool":
                    nc.gpsimd.tensor_copy(ps[:, 0:8, :], C_rz[:])
                    nc.gpsimd.tensor_copy(ps[:, 8:12, :], E_n[:])
                elif PRELOAD == "dve":
                    nc.vector.tensor_copy(ps[:, 0:8, :], C_rz[:])
                    nc.vector.tensor_copy(ps[:, 8:12, :], E_n[:])

            # ---- GRU steps ----
            ps_cur = psg.tile([P, GM, BATCH], F32, tag="gates")
            preload(ps_cur)
            prev = x0_sb
            for t in range(STEPS):
                c, ti = t // TC, t % TC
                use_pre = PRELOAD != "off"
                for m in range(GM):
                    for k in range(KO):
                        nc.tensor.matmul(
                            ps_cur[:, m, :], wih[:, k, m, :], prev[:, k, :],
                            start=(k == 0 and not use_pre), stop=(k == KO - 1),
                        )
                # preload the *next* step's psum early (runs during this chain)
                ps_next = psg.tile([P, GM, BATCH], F32, tag="gates")
                preload(ps_next)

                if use_pre:
                    t_r = sp.tile([P, KO, BATCH], F32, tag="tr")
                    nc.scalar.activation(t_r, ps_cur[:, 0:4, :], AF.Tanh, scale=0.5)
                    t_z = sp.tile([P, KO, BATCH], F32, tag="tz")
                    nc.scalar.activation(t_z, ps_cur[:, 4:8, :], AF.Tanh, scale=0.5)
                else:
                    s_r = sp.tile([P, KO, BATCH], F32, tag="sr")
                    nc.vector.tensor_add(s_r, ps_cur[:, 0:4, :], C_rz[:, 0:4, :])
                    t_r = sp.tile([P, KO, BATCH], F32, tag="tr")
                    nc.scalar.activation(t_r, s_r, AF.Tanh, scale=0.5)
                    s_z = sp.tile([P, KO, BATCH], F32, tag="sz")
                    nc.vector.tensor_add(s_z, ps_cur[:, 4:8, :], C_rz[:, 4:8, :])
                    t_z = sp.tile([P, KO, BATCH], F32, tag="tz")
                    nc.scalar.activation(t_z, s_z, AF.Tanh, scale=0.5)
                # a = tanh(0.5 s_r) * 0.5*gh_n  (the r-gate modulation)
                a = sp.tile([P, KO, BATCH], F32, tag="a")
                nc.gpsimd.tensor_mul(a, t_r, hn2)
                # sn2 = gi_n + E_n + a
                sn2 = sp.tile([P, KO, BATCH], F32, tag="sn2")
                if use_pre:
                    nc.vector.tensor_add(sn2, ps_cur[:, 8:12, :], a)
                else:
                    sn1 = sp.tile([P, KO, BATCH], F32, tag="sn1")
                    nc.vector.tensor_add(sn1, ps_cur[:, 8:12, :], E_n[:])
                    nc.vector.tensor_add(sn2, sn1, a)
                n = sp.tile([P, KO, BATCH], F32, tag="n")
                nc.scalar.activation(n, sn2, AF.Tanh, scale=1.0)
                # h = (0.5 - 0.5 t_z) * n + (h0/2 + 0.5 t_z * h0)
                c1 = sp.tile([P, KO, BATCH], F32, tag="c1")
                nc.gpsimd.scalar_tensor_tensor(
                    c1, t_z, -0.5, halves[:], ALU.mult, ALU.add
                )
                th0 = sp.tile([P, KO, BATCH], F32, tag="th0")
                nc.gpsimd.scalar_tensor_tensor(
                    th0, t_z, 0.5, h0T[:], ALU.mult, ALU.mult
                )
                c2 = sp.tile([P, KO, BATCH], F32, tag="c2")
                nc.gpsimd.tensor_add(c2, th0, h0_half[:])
                m1 = sp.tile([P, KO, BATCH], F32, tag="m1")
                nc.vector.tensor_mul(m1, c1, n)
                dst = res[c][:, :, :, ti]
                nc.vector.tensor_add(dst, m1, c2)
                prev = res[c][:, :, :, ti]
                ps_cur = ps_next

                # interleave projection work for the previous chunk
                emit_pending(UNITS_PER_STEP)
                if ti == TC - 1:
                    pending.extend((c, m) for m in range(MT))

            # drain the last chunk's projection
            emit_pending(len(pending))

    nc.compile()
    return nc


def _shard_inputs(feat, W_hp, b_hp, W_ih, W_hh, b_ih, b_hh, embed, W_out, b_out):
    bf = ml_dtypes.bfloat16
    featT = np.ascontiguousarray(feat.T, dtype=np.float32)
    WhpT = np.ascontiguousarray(W_hp.T, dtype=np.float32)
    WihT = np.ascontiguousarray(W_ih.T).astype(bf)
    WhhT = np.ascontiguousarray(W_hh.T, dtype=np.float32)
    x0T = np.ascontiguousarray(
        np.repeat(np.asarray(embed)[SOS][:, None], BATCH, axis=1)
    ).astype(bf)
    Wo = np.zeros((NCORES * VPAD, HID), np.float32)
    Wo[:VOCAB] = W_out
    bo = np.zeros((NCORES * VPAD,), np.float32)
    bo[:VOCAB] = b_out
    common = dict(
        featT=featT, WhpT=WhpT, WihT=WihT, WhhT=WhhT,
        b_ih=np.asarray(b_ih, np.float32), b_hh=np.asarray(b_hh, np.float32),
        b_hp=np.asarray(b_hp, np.float32), x0T=x0T,
    )
    in_maps = []
    for c in range(NCORES):
        sl = slice(c * VPAD, (c + 1) * VPAD)
        m = dict(common)
        m["WoutT"] = np.ascontiguousarray(Wo[sl].T).astype(bf)
        m["b_out"] = bo[sl].copy()
        in_maps.append(m)
    return in_maps


def kernel(**inputs):
    global LAST_RESULTS
    args = {k: np.asarray(v) for k, v in inputs.items()}
    in_maps = _shard_inputs(
        args["feat"], args["W_hp"], args["b_hp"], args["W_ih"], args["W_hh"],
        args["b_ih"], args["b_hh"], args["embed"], args["W_out"], args["b_out"],
    )
    nc = build()
    res = run_bass_kernel_spmd(nc, in_maps, core_ids=list(range(NCORES)))
    LAST_RESULTS = res
    # OUT per core: [MT, P, NCH, B, TC] -> [B, VPAD, T]
    parts = []
    for r in res.results:
        o = r["OUT"]  # [30, 128, 25, 32, 8]
        o = o.transpose(3, 0, 1, 2, 4).reshape(BATCH, VPAD, STEPS)
        parts.append(o)
    out = np.concatenate(parts, axis=1)[:, :VOCAB, :]
    return np.ascontiguousarray(out, dtype=np.float32)


# revision 17
# speedup vs baseline: 1.7580x; 1.0504x over previous
"""Trainium2 Bass kernel for the GRU caption model.

Computes: h0 = feat @ W_hp.T + b_hp; 200-step GRU with constant hidden-proj
gate pre-activations; logits = outs @ W_out.T + b_out -> [B, V, T].

Strategy: every core runs the (tiny, latency-bound) GRU redundantly; the
vocab dimension of W_out is sharded 8 ways; each core emits its own
[B, 3840, T] logits slice which the host concatenates.

Schedule: GRU steps are grouped into 8-step chunks. While the GRU runs
chunk c+1, the vocab projection for chunk c's hidden states is interleaved
into the PE queue a few matmuls per step, keeping the tensor engine
continuously busy (max p-state) and hiding the recurrence's cross-engine
dependency-chain latency. Output DMAs use 1024B contiguous runs; the host
unscrambles the [MT, 128, NCH, B, TC] layout for free.

The constant gate pre-activations C_rz = gh_rz + b_ih_rz and E_n
(= 0.5*gh_n + b_ih_n) are preloaded into PSUM each step so the gate
matmuls accumulate on top of them (start=False) and tanh reads PSUM
directly, shortening the per-step chain.
"""

import numpy as np
import ml_dtypes

import concourse.bass as bass
import concourse.mybir as mybir
import concourse.tile as tile
from concourse import bacc
from concourse.bass_utils import run_bass_kernel_spmd

F32 = mybir.dt.float32
BF16 = mybir.dt.bfloat16
AF = mybir.ActivationFunctionType
ALU = mybir.AluOpType

VOCAB = 30522
HID = 512
FEAT = 2048
STEPS = 200
BATCH = 32
SOS = 101
NCORES = 8
P = 128
KO = HID // P          # 4 h-chunks
GM = 3 * HID // P      # 12 gate row-groups (r: 0-3, z: 4-7, n: 8-11)
KF = FEAT // P         # 16 feat chunks
VPAD = 3840            # per-core padded vocab rows = 30 * 128
MT = VPAD // P         # 30 vocab tiles per core
TC = 8                 # GRU steps per projection chunk
NCH = STEPS // TC      # 25 chunks
NPROJ = BATCH * TC     # moving free size per proj matmul = 256

LAST_RESULTS = None    # test harness introspection
PRELOAD = "pe"         # 'pe' | 'off' — preload C into PSUM via identity matmul
                       # (non-PE psum writes before a start=True matmul are
                       # rejected by the executor, so the preload must be a
                       # matmul: ps = I128 @ [C_hi; C_lo] in two bf16 passes)
UNITS_PER_STEP = 4     # proj units interleaved per GRU step


def build():
    nc = bacc.Bacc("TRN2", target_bir_lowering=False, debug=False)

    featT = nc.dram_tensor("featT", [FEAT, BATCH], F32, kind="ExternalInput")
    WhpT = nc.dram_tensor("WhpT", [FEAT, HID], F32, kind="ExternalInput")
    WihT = nc.dram_tensor("WihT", [HID, 3 * HID], BF16, kind="ExternalInput")
    WhhT = nc.dram_tensor("WhhT", [HID, 3 * HID], F32, kind="ExternalInput")
    b_ih = nc.dram_tensor("b_ih", [3 * HID], F32, kind="ExternalInput")
    b_hh = nc.dram_tensor("b_hh", [3 * HID], F32, kind="ExternalInput")
    b_hp = nc.dram_tensor("b_hp", [HID], F32, kind="ExternalInput")
    x0T = nc.dram_tensor("x0T", [HID, BATCH], BF16, kind="ExternalInput")
    I128 = nc.dram_tensor("I128", [P, P], BF16, kind="ExternalInput")
    WoutT = nc.dram_tensor("WoutT", [HID, VPAD], BF16, kind="ExternalInput")
    b_out = nc.dram_tensor("b_out", [VPAD], F32, kind="ExternalInput")
    OUT = nc.dram_tensor("OUT", [MT, P, NCH, BATCH, TC], F32, kind="ExternalOutput")

    with tile.TileContext(nc) as tc:
        with (
            tc.tile_pool(name="const", bufs=1) as const,
            tc.tile_pool(name="stream", bufs=3) as stream,
            tc.tile_pool(name="step", bufs=3) as sp,
            tc.tile_pool(name="outp", bufs=6) as outp,
            tc.tile_pool(name="psg", bufs=2, space="PSUM") as psg,
            tc.tile_pool(name="psp", bufs=2, space="PSUM") as psp,
        ):
            # ---- constants into SBUF ----
            featT_sb = const.tile([P, KF, BATCH], F32, tag="featsb")
            nc.sync.dma_start(featT_sb[:], featT.rearrange("(k p) b -> p k b", p=P))
            bih_sb = const.tile([P, GM], F32, tag="bih")
            nc.sync.dma_start(bih_sb[:], b_ih.rearrange("(m p) -> p m", p=P))
            bhh_sb = const.tile([P, GM], F32, tag="bhh")
            nc.sync.dma_start(bhh_sb[:], b_hh.rearrange("(m p) -> p m", p=P))
            bhp_sb = const.tile([P, KO], F32, tag="bhp")
            nc.sync.dma_start(bhp_sb[:], b_hp.rearrange("(m p) -> p m", p=P))
            bout_sb = const.tile([P, MT], F32, tag="bout")
            nc.sync.dma_start(bout_sb[:], b_out.rearrange("(m p) -> p m", p=P))
            halves = const.tile([P, KO, BATCH], F32, tag="halves")
            nc.vector.memset(halves[:], 0.5)
            x0_sb = const.tile([P, KO, BATCH], BF16, tag="x0")
            nc.sync.dma_start(x0_sb[:], x0T.rearrange("(k p) b -> p k b", p=P))
            i128_sb = const.tile([P, P], BF16, tag="i128")
            nc.sync.dma_start(i128_sb[:], I128[:, :])
            wih = const.tile([P, KO, GM, P], BF16, tag="wih")
            nc.sync.dma_start(
                wih[:], WihT.rearrange("(k p) (m c) -> p k m c", p=P, c=P)
            )
            wout = const.tile([P, KO, VPAD], BF16, tag="wout")
            nc.sync.dma_start(wout[:], WoutT.rearrange("(k p) v -> p k v", p=P))

            WhpT_r = WhpT.rearrange("(k p) h -> p k h", p=P)
            WhhT_r = WhhT.rearrange("(k p) g -> p k g", p=P)

            # ---- h0 = feat @ W_hp.T + b_hp (fp32, exact) ----
            ps_h = psg.tile([P, GM, BATCH], F32, tag="gates")
            for ko in range(KO):
                for kf in range(KF):
                    wt = stream.tile([P, P], F32, tag="whp")
                    nc.sync.dma_start(wt[:], WhpT_r[:, kf, ko * P:(ko + 1) * P])
                    nc.tensor.matmul(
                        ps_h[:, ko, :], wt[:], featT_sb[:, kf, :],
                        start=(kf == 0), stop=(kf == KF - 1),
                    )
            h0T = const.tile([P, KO, BATCH], F32, tag="h0T")
            for ko in range(KO):
                nc.scalar.activation(
                    h0T[:, ko, :], ps_h[:, ko, :], AF.Identity,
                    bias=bhp_sb[:, ko, None], scale=1.0,
                )
            h0_half = const.tile([P, KO, BATCH], F32, tag="h0h")
            nc.scalar.mul(h0_half[:], h0T[:], 0.5)

            # ---- gh = h0 @ W_hh.T + b_hh (fp32, exact; step-invariant) ----
            ps_g = psg.tile([P, GM, BATCH], F32, tag="gates")
            for m in range(GM):
                for k in range(KO):
                    wt = stream.tile([P, P], F32, tag="whh")
                    nc.sync.dma_start(wt[:], WhhT_r[:, k, m * P:(m + 1) * P])
                    nc.tensor.matmul(
                        ps_g[:, m, :], wt[:], h0T[:, k, :],
                        start=(k == 0), stop=(k == KO - 1),
                    )
            ghT = const.tile([P, GM, BATCH], F32, tag="ghT")
            for m in range(GM):
                nc.scalar.activation(
                    ghT[:, m, :], ps_g[:, m, :], AF.Identity,
                    bias=bhh_sb[:, m, None], scale=1.0,
                )
            # C_rz = gh_rz + b_ih_rz ; hn2 = 0.5*gh_n ; E_n = hn2 + b_ih_n
            C_rz = const.tile([P, 8, BATCH], F32, tag="Crz")
            nc.vector.tensor_add(
                C_rz[:], ghT[:, 0:8, :],
                bih_sb[:, 0:8, None].to_broadcast((P, 8, BATCH)),
            )
            hn2 = const.tile([P, KO, BATCH], F32, tag="hn2")
            nc.scalar.mul(hn2[:], ghT[:, 8:12, :], 0.5)
            E_n = const.tile([P, KO, BATCH], F32, tag="En")
            nc.vector.tensor_add(
                E_n[:], hn2[:],
                bih_sb[:, 8:12, None].to_broadcast((P, KO, BATCH)),
            )
            # C_all = [C_rz ; E_n] split into bf16 hi+lo for exact PE preload
            C_all = const.tile([P, GM, BATCH], F32, tag="Call")
            nc.vector.tensor_copy(C_all[:, 0:8, :], C_rz[:])
            nc.vector.tensor_copy(C_all[:, 8:12, :], E_n[:])
            C_hi = const.tile([P, GM, BATCH], BF16, tag="Chi")
            nc.vector.tensor_copy(C_hi[:], C_all[:])
            C_hi32 = const.tile([P, GM, BATCH], F32, tag="Chi32")
            nc.scalar.copy(C_hi32[:], C_hi[:])
            C_lo32 = const.tile([P, GM, BATCH], F32, tag="Clo32")
            nc.vector.tensor_sub(C_lo32[:], C_all[:], C_hi32[:])
            C_lo = const.tile([P, GM, BATCH], BF16, tag="Clo")
            nc.vector.tensor_copy(C_lo[:], C_lo32[:])

            # hidden-state history, bf16, chunked by TC steps
            res = [
                const.tile([P, KO, BATCH, TC], BF16, tag=f"res{c}", name=f"res{c}")
                for c in range(NCH)
            ]

            # ---- projection unit: one vocab tile m of chunk c ----
            def proj_unit(c, m):
                ps2 = psp.tile([P, NPROJ], F32, tag="pp", name="pp")
                for k in range(KO):
                    nc.tensor.matmul(
                        ps2, wout[:, k, m * P:(m + 1) * P], res[c][:, k, :, :],
                        start=(k == 0), stop=(k == KO - 1),
                    )
                ob = outp.tile([P, NPROJ], F32, tag="ob", name="ob")
                nc.gpsimd.tensor_scalar_add(ob, ps2, bout_sb[:, m, None])
                nc.sync.dma_start(
                    OUT[m, :, c, :, :], ob.rearrange("p (b t) -> p b t", b=BATCH)
                )

            pending = []     # (c, m) proj units ready to emit
            pend_i = 0

            def emit_pending(limit):
                nonlocal pend_i
                done = 0
                while pend_i < len(pending) and done < limit:
                    proj_unit(*pending[pend_i])
                    pend_i += 1
                    done += 1

            def preload(ps):
                if PRELOAD == "pe":
                    flat = ps[:, :, :].rearrange("p m b -> p (m b)")
                    nc.tensor.matmul(
                        flat, i128_sb[:],
                        C_hi[:].rearrange("p m b -> p (m b)"),
                        start=True, stop=False,
                    )
                    nc.tensor.matmul(
                        flat, i128_sb[:],
                        C_lo[:].rearrange("p m b -> p (m b)"),
                        start=False, stop=False,
                    )

            # ---- GRU steps ----
            ps_cur = psg.tile([P, GM, BATCH], F32, tag="gates")
            preload(ps_cur)
            prev = x0_sb
            for t in range(STEPS):
                c, ti = t // TC, t % TC
                use_pre = PRELOAD != "off"
                for m in range(GM):
                    for k in range(KO):
                        nc.tensor.matmul(
                            ps_cur[:, m, :], wih[:, k, m, :], prev[:, k, :],
                            start=(k == 0 and not use_pre),
                            stop=(k == KO - 1 and (not use_pre or m == GM - 1)),
                        )
                # preload the *next* step's psum early (runs during this chain)
                ps_next = psg.tile([P, GM, BATCH], F32, tag="gates")
                preload(ps_next)

                if use_pre:
                    t_r = sp.tile([P, KO, BATCH], F32, tag="tr")
                    nc.scalar.activation(t_r, ps_cur[:, 0:4, :], AF.Tanh, scale=0.5)
                    t_z = sp.tile([P, KO, BATCH], F32, tag="tz")
                    nc.scalar.activation(t_z, ps_cur[:, 4:8, :], AF.Tanh, scale=0.5)
                else:
                    s_r = sp.tile([P, KO, BATCH], F32, tag="sr")
                    nc.vector.tensor_add(s_r, ps_cur[:, 0:4, :], C_rz[:, 0:4, :])
                    t_r = sp.tile([P, KO, BATCH], F32, tag="tr")
                    nc.scalar.activation(t_r, s_r, AF.Tanh, scale=0.5)
                    s_z = sp.tile([P, KO, BATCH], F32, tag="sz")
                    nc.vector.tensor_add(s_z, ps_cur[:, 4:8, :], C_rz[:, 4:8, :])
                    t_z = sp.tile([P, KO, BATCH], F32, tag="tz")
                    nc.scalar.activation(t_z, s_z, AF.Tanh, scale=0.5)
                # a = tanh(0.5 s_r) * 0.5*gh_n  (the r-gate modulation)
                a = sp.tile([P, KO, BATCH], F32, tag="a")
                nc.gpsimd.tensor_mul(a, t_r, hn2)
                # sn2 = gi_n + E_n + a
                sn2 = sp.tile([P, KO, BATCH], F32, tag="sn2")
                if use_pre:
                    nc.vector.tensor_add(sn2, ps_cur[:, 8:12, :], a)
                else:
                    sn1 = sp.tile([P, KO, BATCH], F32, tag="sn1")
                    nc.vector.tensor_add(sn1, ps_cur[:, 8:12, :], E_n[:])
                    nc.vector.tensor_add(sn2, sn1, a)
                n = sp.tile([P, KO, BATCH], F32, tag="n")
                nc.scalar.activation(n, sn2, AF.Tanh, scale=1.0)
                # h = (0.5 - 0.5 t_z) * n + (h0/2 + 0.5 t_z * h0)
                c1 = sp.tile([P, KO, BATCH], F32, tag="c1")
                nc.gpsimd.scalar_tensor_tensor(
                    c1, t_z, -0.5, halves[:], ALU.mult, ALU.add
                )
                th0 = sp.tile([P, KO, BATCH], F32, tag="th0")
                nc.gpsimd.scalar_tensor_tensor(
                    th0, t_z, 0.5, h0T[:], ALU.mult, ALU.mult
                )
                c2 = sp.tile([P, KO, BATCH], F32, tag="c2")
                nc.gpsimd.tensor_add(c2, th0, h0_half[:])
                m1 = sp.tile([P, KO, BATCH], F32, tag="m1")
                nc.vector.tensor_mul(m1, c1, n)
                dst = res[c][:, :, :, ti]
                nc.vector.tensor_add(dst, m1, c2)
                prev = res[c][:, :, :, ti]
                ps_cur = ps_next

                # interleave projection work for the previous chunk
                emit_pending(UNITS_PER_STEP)
                if ti == TC - 1:
                    pending.extend((c, m) for m in range(MT))

            # drain the last chunk's projection
            emit_pending(len(pending))

    nc.compile()
    return nc


def _shard_inputs(feat, W_hp, b_hp, W_ih, W_hh, b_ih, b_hh, embed, W_out, b_out):
    bf = ml_dtypes.bfloat16
    featT = np.ascontiguousarray(feat.T, dtype=np.float32)
    WhpT = np.ascontiguousarray(W_hp.T, dtype=np.float32)
    WihT = np.ascontiguousarray(W_ih.T).astype(bf)
    WhhT = np.ascontiguousarray(W_hh.T, dtype=np.float32)
    x0T = np.ascontiguousarray(
        np.repeat(np.asarray(embed)[SOS][:, None], BATCH, axis=1)
    ).astype(bf)
    Wo = np.zeros((NCORES * VPAD, HID), np.float32)
    Wo[:VOCAB] = W_out
    bo = np.zeros((NCORES * VPAD,), np.float32)
    bo[:VOCAB] = b_out
    common = dict(
        featT=featT, WhpT=WhpT, WihT=WihT, WhhT=WhhT,
        b_ih=np.asarray(b_ih, np.float32), b_hh=np.asarray(b_hh, np.float32),
        b_hp=np.asarray(b_hp, np.float32), x0T=x0T,
        I128=np.eye(P, dtype=np.float32).astype(bf),
    )
    in_maps = []
    for c in range(NCORES):
        sl = slice(c * VPAD, (c + 1) * VPAD)
        m = dict(common)
        m["WoutT"] = np.ascontiguousarray(Wo[sl].T).astype(bf)
        m["b_out"] = bo[sl].copy()
        in_maps.append(m)
    return in_maps


def kernel(**inputs):
    global LAST_RESULTS
    args = {k: np.asarray(v) for k, v in inputs.items()}
    in_maps = _shard_inputs(
        args["feat"], args["W_hp"], args["b_hp"], args["W_ih"], args["W_hh"],
        args["b_ih"], args["b_hh"], args["embed"], args["W_out"], args["b_out"],
    )
    nc = build()
    res = run_bass_kernel_spmd(nc, in_maps, core_ids=list(range(NCORES)))
    LAST_RESULTS = res
    # OUT per core: [MT, P, NCH, B, TC] -> [B, VPAD, T]
    parts = []
    for r in res.results:
        o = r["OUT"]  # [30, 128, 25, 32, 8]
        o = o.transpose(3, 0, 1, 2, 4).reshape(BATCH, VPAD, STEPS)
        parts.append(o)
    out = np.concatenate(parts, axis=1)[:, :VOCAB, :]
    return np.ascontiguousarray(out, dtype=np.float32)


# revision 23
# speedup vs baseline: 1.9094x; 1.0861x over previous
"""Trainium2 Bass kernel for the GRU caption model.

Computes: h0 = feat @ W_hp.T + b_hp; 200-step GRU with constant hidden-proj
gate pre-activations; logits = outs @ W_out.T + b_out -> [B, V, T].

Strategy: every core runs the (tiny, latency-bound) GRU redundantly; the
vocab dimension of W_out is sharded 8 ways; each core emits its own
[B, 3840, T] logits slice which the host concatenates.

Schedule: GRU steps are grouped into 8-step chunks. While the GRU runs
chunk c+1, the vocab projection for chunk c's hidden states is interleaved
into the PE queue a few matmuls per step, keeping the tensor engine
continuously busy (max p-state) and hiding the recurrence's cross-engine
dependency-chain latency. Output DMAs use 1024B contiguous runs; the host
unscrambles the [MT, 128, NCH, B, TC] layout for free.

The constant gate pre-activations (C_rz = gh_rz + b_ih_rz and
E_n = 0.5*gh_n + b_ih_n) are preloaded into PSUM via identity matmuls
(bf16 hi+lo, exact to ~2^-17) so the gate matmuls accumulate on top of
them and tanh reads PSUM directly. The r/z/n gate groups live in three
bank-aligned PSUM tiles so the r-gate tanh fires as soon as the r matmuls
land instead of waiting for the whole gate tile.
"""

import numpy as np
import ml_dtypes

import concourse.bass as bass
import concourse.mybir as mybir
import concourse.tile as tile
from concourse import bacc
from concourse.bass_utils import run_bass_kernel_spmd

F32 = mybir.dt.float32
BF16 = mybir.dt.bfloat16
AF = mybir.ActivationFunctionType
ALU = mybir.AluOpType

VOCAB = 30522
HID = 512
FEAT = 2048
STEPS = 200
BATCH = 32
SOS = 101
NCORES = 8
P = 128
KO = HID // P          # 4 h-chunks
GM = 3 * HID // P      # 12 gate row-groups (r: 0-3, z: 4-7, n: 8-11)
KF = FEAT // P         # 16 feat chunks
VPAD = 3840            # per-core padded vocab rows = 30 * 128
MT = VPAD // P         # 30 vocab tiles per core
TC = 8                 # GRU steps per projection chunk
NCH = STEPS // TC      # 25 chunks
NPROJ = BATCH * TC     # moving free size per proj matmul = 256

LAST_RESULTS = None    # test harness introspection
UNITS_PER_STEP = 4     # proj units interleaved per GRU step


def build():
    nc = bacc.Bacc("TRN2", target_bir_lowering=False, debug=False)

    featT = nc.dram_tensor("featT", [FEAT, BATCH], F32, kind="ExternalInput")
    WhpT = nc.dram_tensor("WhpT", [FEAT, HID], F32, kind="ExternalInput")
    WihT = nc.dram_tensor("WihT", [HID, 3 * HID], BF16, kind="ExternalInput")
    WhhT = nc.dram_tensor("WhhT", [HID, 3 * HID], F32, kind="ExternalInput")
    b_ih = nc.dram_tensor("b_ih", [3 * HID], F32, kind="ExternalInput")
    b_hh = nc.dram_tensor("b_hh", [3 * HID], F32, kind="ExternalInput")
    b_hp = nc.dram_tensor("b_hp", [HID], F32, kind="ExternalInput")
    x0T = nc.dram_tensor("x0T", [HID, BATCH], BF16, kind="ExternalInput")
    I128 = nc.dram_tensor("I128", [P, P], BF16, kind="ExternalInput")
    WoutT = nc.dram_tensor("WoutT", [HID, VPAD], BF16, kind="ExternalInput")
    b_out = nc.dram_tensor("b_out", [VPAD], F32, kind="ExternalInput")
    OUT = nc.dram_tensor("OUT", [MT, P, NCH, BATCH, TC], F32, kind="ExternalOutput")

    with tile.TileContext(nc) as tc:
        with (
            tc.tile_pool(name="const", bufs=1) as const,
            tc.tile_pool(name="stream", bufs=3) as stream,
            tc.tile_pool(name="step", bufs=3) as sp,
            tc.tile_pool(name="outp", bufs=6) as outp,
            tc.tile_pool(name="psg", bufs=2, space="PSUM") as psg,
            tc.tile_pool(name="psp", bufs=2, space="PSUM") as psp,
        ):
            # ---- constants into SBUF (order = DMA queue order; the h0/gh
            # weight streams are emitted inside the loops below, and the big
            # wih/wout loads are deferred until after them so they don't
            # block the startup-critical transfers) ----
            featT_sb = const.tile([P, KF, BATCH], F32, tag="featsb")
            nc.sync.dma_start(featT_sb[:], featT.rearrange("(k p) b -> p k b", p=P))
            bih_sb = const.tile([P, GM], F32, tag="bih")
            nc.sync.dma_start(bih_sb[:], b_ih.rearrange("(m p) -> p m", p=P))
            bhh_sb = const.tile([P, GM], F32, tag="bhh")
            nc.sync.dma_start(bhh_sb[:], b_hh.rearrange("(m p) -> p m", p=P))
            bhp_sb = const.tile([P, KO], F32, tag="bhp")
            nc.sync.dma_start(bhp_sb[:], b_hp.rearrange("(m p) -> p m", p=P))
            x0_sb = const.tile([P, KO, BATCH], BF16, tag="x0")
            nc.sync.dma_start(x0_sb[:], x0T.rearrange("(k p) b -> p k b", p=P))
            i128_sb = const.tile([P, P], BF16, tag="i128")
            nc.sync.dma_start(i128_sb[:], I128[:, :])
            halves = const.tile([P, KO, BATCH], F32, tag="halves")
            nc.vector.memset(halves[:], 0.5)

            WhpT_r = WhpT.rearrange("(k p) h -> p k h", p=P)
            WhhT_r = WhhT.rearrange("(k p) g -> p k g", p=P)

            # ---- h0 = feat @ W_hp.T + b_hp (fp32, exact) ----
            ps_h0 = psg.tile([P, 512], F32, tag="psr", name="psr")
            ps_h = ps_h0[:, 0 : KO * BATCH].rearrange("p (m b) -> p m b", b=BATCH)
            for ko in range(KO):
                for kf in range(KF):
                    wt = stream.tile([P, P], F32, tag="whp")
                    nc.sync.dma_start(wt[:], WhpT_r[:, kf, ko * P:(ko + 1) * P])
                    nc.tensor.matmul(
                        ps_h[:, ko, :], wt[:], featT_sb[:, kf, :],
                        start=(kf == 0), stop=(kf == KF - 1),
                    )
            h0T = const.tile([P, KO, BATCH], F32, tag="h0T")
            for ko in range(KO):
                nc.scalar.activation(
                    h0T[:, ko, :], ps_h[:, ko, :], AF.Identity,
                    bias=bhp_sb[:, ko, None], scale=1.0,
                )
            h0_half = const.tile([P, KO, BATCH], F32, tag="h0h")
            nc.scalar.mul(h0_half[:], h0T[:], 0.5)

            # ---- gh = h0 @ W_hh.T + b_hh (fp32, exact; step-invariant) ----
            ghT = const.tile([P, GM, BATCH], F32, tag="ghT")
            for part, tg in ((0, "psz"), (1, "psn"), (2, "psr")):
                ps_g0 = psg.tile([P, 512], F32, tag=tg, name=tg)
                ps_g = ps_g0[:, 0 : KO * BATCH].rearrange("p (m b) -> p m b", b=BATCH)
                for mi in range(KO):
                    m = part * KO + mi
                    for k in range(KO):
                        wt = stream.tile([P, P], F32, tag="whh")
                        nc.sync.dma_start(wt[:], WhhT_r[:, k, m * P:(m + 1) * P])
                        nc.tensor.matmul(
                            ps_g[:, mi, :], wt[:], h0T[:, k, :],
                            start=(k == 0), stop=(k == KO - 1),
                        )
                for mi in range(KO):
                    m = part * KO + mi
                    nc.scalar.activation(
                        ghT[:, m, :], ps_g[:, mi, :], AF.Identity,
                        bias=bhh_sb[:, m, None], scale=1.0,
                    )

            # big resident weights (loaded while h0/gh compute)
            wih = const.tile([P, KO, GM, P], BF16, tag="wih")
            nc.sync.dma_start(
                wih[:], WihT.rearrange("(k p) (m c) -> p k m c", p=P, c=P)
            )
            bout_sb = const.tile([P, MT], F32, tag="bout")
            nc.sync.dma_start(bout_sb[:], b_out.rearrange("(m p) -> p m", p=P))
            wout = const.tile([P, KO, VPAD], BF16, tag="wout")
            nc.sync.dma_start(wout[:], WoutT.rearrange("(k p) v -> p k v", p=P))

            # C_all = [C_rz ; E_n]: constant additive gate pre-activations
            C_all = const.tile([P, GM, BATCH], F32, tag="Call")
            nc.vector.tensor_add(
                C_all[:, 0:8, :], ghT[:, 0:8, :],
                bih_sb[:, 0:8, None].to_broadcast((P, 8, BATCH)),
            )
            hn2 = const.tile([P, KO, BATCH], F32, tag="hn2")
            nc.scalar.mul(hn2[:], ghT[:, 8:12, :], 0.5)
            nc.vector.tensor_add(
                C_all[:, 8:12, :], hn2[:],
                bih_sb[:, 8:12, None].to_broadcast((P, KO, BATCH)),
            )
            # split into bf16 hi+lo for exact PE psum preload
            C_hi = const.tile([P, GM, BATCH], BF16, tag="Chi")
            nc.vector.tensor_copy(C_hi[:], C_all[:])
            C_hi32 = const.tile([P, GM, BATCH], F32, tag="Chi32")
            nc.scalar.copy(C_hi32[:], C_hi[:])
            C_lo32 = const.tile([P, GM, BATCH], F32, tag="Clo32")
            nc.vector.tensor_sub(C_lo32[:], C_all[:], C_hi32[:])
            C_lo = const.tile([P, GM, BATCH], BF16, tag="Clo")
            nc.vector.tensor_copy(C_lo[:], C_lo32[:])

            # hidden-state history, bf16, chunked by TC steps
            res = [
                const.tile([P, KO, BATCH, TC], BF16, tag=f"res{c}", name=f"res{c}")
                for c in range(NCH)
            ]

            # ---- projection unit: one vocab tile m of chunk c ----
            def proj_unit(c, m):
                ps2 = psp.tile([P, NPROJ], F32, tag="pp", name="pp")
                for k in range(KO):
                    nc.tensor.matmul(
                        ps2, wout[:, k, m * P:(m + 1) * P], res[c][:, k, :, :],
                        start=(k == 0), stop=(k == KO - 1),
                    )
                ob = outp.tile([P, NPROJ], F32, tag="ob", name="ob")
                nc.gpsimd.tensor_scalar_add(ob, ps2, bout_sb[:, m, None])
                nc.sync.dma_start(
                    OUT[m, :, c, :, :], ob.rearrange("p (b t) -> p b t", b=BATCH)
                )

            pending = []     # (c, m) proj units ready to emit
            pend_i = 0

            def emit_pending(limit):
                nonlocal pend_i
                done = 0
                while pend_i < len(pending) and done < limit:
                    proj_unit(*pending[pend_i])
                    pend_i += 1
                    done += 1

            def gate_psums():
                """Allocate r/z/n psum tiles and preload C (start=True)."""
                tiles = []
                for i, tg in enumerate(("psr", "psz", "psn")):
                    ps = psg.tile([P, 512], F32, tag=tg, name=tg)
                    flat = ps[:, 0 : KO * BATCH]
                    sl = slice(i * KO, (i + 1) * KO)
                    nc.tensor.matmul(
                        flat, i128_sb[:], C_hi[:, sl, :], start=True, stop=False
                    )
                    nc.tensor.matmul(
                        flat, i128_sb[:], C_lo[:, sl, :], start=False, stop=False
                    )
                    tiles.append(flat.rearrange("p (m b) -> p m b", b=BATCH))
                return tiles

            # ---- GRU steps ----
            cur = gate_psums()
            prev = x0_sb
            for t in range(STEPS):
                c, ti = t // TC, t % TC
                for m in range(GM):
                    ps = cur[m // KO]
                    for k in range(KO):
                        nc.tensor.matmul(
                            ps[:, m % KO, :], wih[:, k, m, :], prev[:, k, :],
                            start=False,
                            stop=(k == KO - 1 and m % KO == KO - 1),
                        )
                ps_r, ps_z, ps_n = cur
                nxt = gate_psums()  # preload next step early (no deps on h)

                t_r = sp.tile([P, KO, BATCH], F32, tag="tr")
                nc.scalar.activation(t_r, ps_r[:, :, :], AF.Tanh, scale=0.5)
                t_z = sp.tile([P, KO, BATCH], F32, tag="tz")
                nc.scalar.activation(t_z, ps_z[:, :, :], AF.Tanh, scale=0.5)
                # a = tanh(0.5 s_r) * 0.5*gh_n  (the r-gate modulation)
                a = sp.tile([P, KO, BATCH], F32, tag="a")
                nc.gpsimd.tensor_mul(a, t_r, hn2)
                # sn2 = gi_n + E_n + a   (E_n already in psum)
                sn2 = sp.tile([P, KO, BATCH], F32, tag="sn2")
                nc.vector.tensor_add(sn2, ps_n[:, :, :], a)
                n = sp.tile([P, KO, BATCH], F32, tag="n")
                nc.scalar.activation(n, sn2, AF.Tanh, scale=1.0)
                # h = (0.5 - 0.5 t_z) * n + (h0/2 + 0.5 t_z * h0)
                c1 = sp.tile([P, KO, BATCH], F32, tag="c1")
                nc.gpsimd.scalar_tensor_tensor(
                    c1, t_z, -0.5, halves[:], ALU.mult, ALU.add
                )
                th0 = sp.tile([P, KO, BATCH], F32, tag="th0")
                nc.gpsimd.scalar_tensor_tensor(
                    th0, t_z, 0.5, h0T[:], ALU.mult, ALU.mult
                )
                c2 = sp.tile([P, KO, BATCH], F32, tag="c2")
                nc.gpsimd.tensor_add(c2, th0, h0_half[:])
                m1 = sp.tile([P, KO, BATCH], F32, tag="m1")
                nc.vector.tensor_mul(m1, c1, n)
                dst = res[c][:, :, :, ti]
                nc.vector.tensor_add(dst, m1, c2)
                prev = res[c][:, :, :, ti]
                cur = nxt

                # interleave projection work for the previous chunk
                emit_pending(UNITS_PER_STEP)
                if ti == TC - 1:
                    pending.extend((c, m) for m in range(MT))

            # drain the last chunk's projection
            emit_pending(len(pending))

    nc.compile()
    return nc


def _shard_inputs(feat, W_hp, b_hp, W_ih, W_hh, b_ih, b_hh, embed, W_out, b_out):
    bf = ml_dtypes.bfloat16
    featT = np.ascontiguousarray(feat.T, dtype=np.float32)
    WhpT = np.ascontiguousarray(W_hp.T, dtype=np.float32)
    WihT = np.ascontiguousarray(W_ih.T).astype(bf)
    WhhT = np.ascontiguousarray(W_hh.T, dtype=np.float32)
    x0T = np.ascontiguousarray(
        np.repeat(np.asarray(embed)[SOS][:, None], BATCH, axis=1)
    ).astype(bf)
    Wo = np.zeros((NCORES * VPAD, HID), np.float32)
    Wo[:VOCAB] = W_out
    bo = np.zeros((NCORES * VPAD,), np.float32)
    bo[:VOCAB] = b_out
    common = dict(
        featT=featT, WhpT=WhpT, WihT=WihT, WhhT=WhhT,
        b_ih=np.asarray(b_ih, np.float32), b_hh=np.asarray(b_hh, np.float32),
        b_hp=np.asarray(b_hp, np.float32), x0T=x0T,
        I128=np.eye(P, dtype=np.float32).astype(bf),
    )
    in_maps = []
    for c in range(NCORES):
        sl = slice(c * VPAD, (c + 1) * VPAD)
        m = dict(common)
        m["WoutT"] = np.ascontiguousarray(Wo[sl].T).astype(bf)
        m["b_out"] = bo[sl].copy()
        in_maps.append(m)
    return in_maps


def kernel(**inputs):
    global LAST_RESULTS
    args = {k: np.asarray(v) for k, v in inputs.items()}
    in_maps = _shard_inputs(
        args["feat"], args["W_hp"], args["b_hp"], args["W_ih"], args["W_hh"],
        args["b_ih"], args["b_hh"], args["embed"], args["W_out"], args["b_out"],
    )
    nc = build()
    res = run_bass_kernel_spmd(nc, in_maps, core_ids=list(range(NCORES)))
    LAST_RESULTS = res
    # OUT per core: [MT, P, NCH, B, TC] -> [B, VPAD, T]
    parts = []
    for r in res.results:
        o = r["OUT"]  # [30, 128, 25, 32, 8]
        o = o.transpose(3, 0, 1, 2, 4).reshape(BATCH, VPAD, STEPS)
        parts.append(o)
    out = np.concatenate(parts, axis=1)[:, :VOCAB, :]
    return np.ascontiguousarray(out, dtype=np.float32)


# revision 25
# speedup vs baseline: 2.1694x; 1.1361x over previous
"""Trainium2 Bass kernel for the GRU caption model.

Computes: h0 = feat @ W_hp.T + b_hp; 200-step GRU with constant hidden-proj
gate pre-activations; logits = outs @ W_out.T + b_out -> [B, V, T].

Strategy: every core runs the (tiny, latency-bound) GRU redundantly; the
vocab dimension of W_out is sharded 8 ways; each core emits its own
[B, 3840, T] logits slice which the host concatenates.

Schedule: GRU steps are grouped into 8-step chunks. While the GRU runs
chunk c+1, the vocab projection for chunk c's hidden states is interleaved
into the PE queue a few matmuls per step, keeping the tensor engine
continuously busy (max p-state) and hiding the recurrence's cross-engine
dependency-chain latency. Output DMAs use 1024B contiguous runs; the host
unscrambles the [MT, 128, NCH, B, TC] layout for free.

The constant gate pre-activations (C_rz = gh_rz + b_ih_rz and
E_n = 0.5*gh_n + b_ih_n) are preloaded into PSUM via identity matmuls
(bf16 hi+lo, exact to ~2^-17) so the gate matmuls accumulate on top of
them and tanh reads PSUM directly. The r/z/n gate groups live in three
bank-aligned PSUM tiles so the r-gate tanh fires as soon as the r matmuls
land instead of waiting for the whole gate tile.
"""

import numpy as np
import ml_dtypes

import concourse.bass as bass
import concourse.mybir as mybir
import concourse.tile as tile
from concourse import bacc
from concourse.bass_utils import run_bass_kernel_spmd

F32 = mybir.dt.float32
BF16 = mybir.dt.bfloat16
AF = mybir.ActivationFunctionType
ALU = mybir.AluOpType

VOCAB = 30522
HID = 512
FEAT = 2048
STEPS = 200
BATCH = 32
SOS = 101
NCORES = 8
P = 128
KO = HID // P          # 4 h-chunks
GM = 3 * HID // P      # 12 gate row-groups (r: 0-3, z: 4-7, n: 8-11)
KF = FEAT // P         # 16 feat chunks
VPAD = 3840            # per-core padded vocab rows = 30 * 128
MT = VPAD // P         # 30 vocab tiles per core
TC = 8                 # GRU steps per projection chunk
NCH = STEPS // TC      # 25 chunks
NPROJ = BATCH * TC     # moving free size per proj matmul = 256

LAST_RESULTS = None    # test harness introspection
UNITS_PER_STEP = 4     # proj units interleaved per GRU step


def build():
    nc = bacc.Bacc("TRN2", target_bir_lowering=False, debug=False)

    featT = nc.dram_tensor("featT", [FEAT, BATCH], F32, kind="ExternalInput")
    WhpT = nc.dram_tensor("WhpT", [FEAT, HID], F32, kind="ExternalInput")
    WihT = nc.dram_tensor("WihT", [HID, 3 * HID], BF16, kind="ExternalInput")
    WhhT = nc.dram_tensor("WhhT", [HID, 3 * HID], F32, kind="ExternalInput")
    b_ih = nc.dram_tensor("b_ih", [3 * HID], F32, kind="ExternalInput")
    b_hh = nc.dram_tensor("b_hh", [3 * HID], F32, kind="ExternalInput")
    b_hp = nc.dram_tensor("b_hp", [HID], F32, kind="ExternalInput")
    x0T = nc.dram_tensor("x0T", [HID, BATCH], BF16, kind="ExternalInput")
    I128 = nc.dram_tensor("I128", [P, P], BF16, kind="ExternalInput")
    WoutT = nc.dram_tensor("WoutT", [HID, VPAD], BF16, kind="ExternalInput")
    b_out = nc.dram_tensor("b_out", [VPAD], F32, kind="ExternalInput")
    OUT = nc.dram_tensor("OUT", [MT, P, NCH, BATCH, TC], F32, kind="ExternalOutput")

    with tile.TileContext(nc) as tc:
        with (
            tc.tile_pool(name="const", bufs=1) as const,
            tc.tile_pool(name="stream", bufs=3) as stream,
            tc.tile_pool(name="step", bufs=3) as sp,
            tc.tile_pool(name="outp", bufs=6) as outp,
            tc.tile_pool(name="psg", bufs=2, space="PSUM") as psg,
            tc.tile_pool(name="psp", bufs=2, space="PSUM") as psp,
        ):
            # ---- constants into SBUF (order = DMA queue order; the h0/gh
            # weight streams are emitted inside the loops below, and the big
            # wih/wout loads are deferred until after them so they don't
            # block the startup-critical transfers) ----
            featT_sb = const.tile([P, KF, BATCH], F32, tag="featsb")
            nc.sync.dma_start(featT_sb[:], featT.rearrange("(k p) b -> p k b", p=P))
            bih_sb = const.tile([P, GM], F32, tag="bih")
            nc.sync.dma_start(bih_sb[:], b_ih.rearrange("(m p) -> p m", p=P))
            bhh_sb = const.tile([P, GM], F32, tag="bhh")
            nc.sync.dma_start(bhh_sb[:], b_hh.rearrange("(m p) -> p m", p=P))
            bhp_sb = const.tile([P, KO], F32, tag="bhp")
            nc.sync.dma_start(bhp_sb[:], b_hp.rearrange("(m p) -> p m", p=P))
            x0_sb = const.tile([P, KO, BATCH], BF16, tag="x0")
            nc.sync.dma_start(x0_sb[:], x0T.rearrange("(k p) b -> p k b", p=P))
            i128_sb = const.tile([P, P], BF16, tag="i128")
            nc.sync.dma_start(i128_sb[:], I128[:, :])
            halves = const.tile([P, KO, BATCH], F32, tag="halves")
            nc.vector.memset(halves[:], 0.5)

            whp_sb = const.tile([P, KF, HID], F32, tag="whp")
            nc.sync.dma_start(whp_sb[:], WhpT.rearrange("(k p) h -> p k h", p=P))
            whh_sb = const.tile([P, KO, 3 * HID], F32, tag="whh")
            nc.sync.dma_start(whh_sb[:], WhhT.rearrange("(k p) g -> p k g", p=P))

            # ---- h0 = feat @ W_hp.T + b_hp (fp32, exact) ----
            ps_h0 = psg.tile([P, 512], F32, tag="psr", name="psr")
            ps_h = ps_h0[:, 0 : KO * BATCH].rearrange("p (m b) -> p m b", b=BATCH)
            for ko in range(KO):
                for kf in range(KF):
                    nc.tensor.matmul(
                        ps_h[:, ko, :], whp_sb[:, kf, ko * P:(ko + 1) * P],
                        featT_sb[:, kf, :],
                        start=(kf == 0), stop=(kf == KF - 1),
                    )
            h0T = const.tile([P, KO, BATCH], F32, tag="h0T")
            for ko in range(KO):
                nc.scalar.activation(
                    h0T[:, ko, :], ps_h[:, ko, :], AF.Identity,
                    bias=bhp_sb[:, ko, None], scale=1.0,
                )
            h0_half = const.tile([P, KO, BATCH], F32, tag="h0h")
            nc.scalar.mul(h0_half[:], h0T[:], 0.5)

            # ---- gh = h0 @ W_hh.T + b_hh (fp32, exact; step-invariant) ----
            ghT = const.tile([P, GM, BATCH], F32, tag="ghT")
            for part, tg in ((0, "psz"), (1, "psn"), (2, "psr")):
                ps_g0 = psg.tile([P, 512], F32, tag=tg, name=tg)
                ps_g = ps_g0[:, 0 : KO * BATCH].rearrange("p (m b) -> p m b", b=BATCH)
                for mi in range(KO):
                    m = part * KO + mi
                    for k in range(KO):
                        nc.tensor.matmul(
                            ps_g[:, mi, :], whh_sb[:, k, m * P:(m + 1) * P],
                            h0T[:, k, :],
                            start=(k == 0), stop=(k == KO - 1),
                        )
                for mi in range(KO):
                    m = part * KO + mi
                    nc.scalar.activation(
                        ghT[:, m, :], ps_g[:, mi, :], AF.Identity,
                        bias=bhh_sb[:, m, None], scale=1.0,
                    )

            # big resident weights (loaded while h0/gh compute)
            wih = const.tile([P, KO, GM, P], BF16, tag="wih")
            nc.sync.dma_start(
                wih[:], WihT.rearrange("(k p) (m c) -> p k m c", p=P, c=P)
            )
            bout_sb = const.tile([P, MT], F32, tag="bout")
            nc.sync.dma_start(bout_sb[:], b_out.rearrange("(m p) -> p m", p=P))
            wout = const.tile([P, KO, VPAD], BF16, tag="wout")
            nc.sync.dma_start(wout[:], WoutT.rearrange("(k p) v -> p k v", p=P))

            # C_all = [C_rz ; E_n]: constant additive gate pre-activations
            C_all = const.tile([P, GM, BATCH], F32, tag="Call")
            nc.vector.tensor_add(
                C_all[:, 0:8, :], ghT[:, 0:8, :],
                bih_sb[:, 0:8, None].to_broadcast((P, 8, BATCH)),
            )
            hn2 = const.tile([P, KO, BATCH], F32, tag="hn2")
            nc.scalar.mul(hn2[:], ghT[:, 8:12, :], 0.5)
            nc.vector.tensor_add(
                C_all[:, 8:12, :], hn2[:],
                bih_sb[:, 8:12, None].to_broadcast((P, KO, BATCH)),
            )
            # split into bf16 hi+lo for exact PE psum preload
            C_hi = const.tile([P, GM, BATCH], BF16, tag="Chi")
            nc.vector.tensor_copy(C_hi[:], C_all[:])
            C_hi32 = const.tile([P, GM, BATCH], F32, tag="Chi32")
            nc.scalar.copy(C_hi32[:], C_hi[:])
            C_lo32 = const.tile([P, GM, BATCH], F32, tag="Clo32")
            nc.vector.tensor_sub(C_lo32[:], C_all[:], C_hi32[:])
            C_lo = const.tile([P, GM, BATCH], BF16, tag="Clo")
            nc.vector.tensor_copy(C_lo[:], C_lo32[:])

            # hidden-state history, bf16, chunked by TC steps
            res = [
                const.tile([P, KO, BATCH, TC], BF16, tag=f"res{c}", name=f"res{c}")
                for c in range(NCH)
            ]

            # ---- projection unit: one vocab tile m of chunk c ----
            def proj_unit(c, m):
                ps2 = psp.tile([P, NPROJ], F32, tag="pp", name="pp")
                for k in range(KO):
                    nc.tensor.matmul(
                        ps2, wout[:, k, m * P:(m + 1) * P], res[c][:, k, :, :],
                        start=(k == 0), stop=(k == KO - 1),
                    )
                ob = outp.tile([P, NPROJ], F32, tag="ob", name="ob")
                nc.gpsimd.tensor_scalar_add(ob, ps2, bout_sb[:, m, None])
                nc.sync.dma_start(
                    OUT[m, :, c, :, :], ob.rearrange("p (b t) -> p b t", b=BATCH)
                )

            pending = []     # (c, m) proj units ready to emit
            pend_i = 0

            def emit_pending(limit):
                nonlocal pend_i
                done = 0
                while pend_i < len(pending) and done < limit:
                    proj_unit(*pending[pend_i])
                    pend_i += 1
                    done += 1

            def gate_psums():
                """Allocate r/z/n psum tiles and preload C (start=True)."""
                tiles = []
                for i, tg in enumerate(("psr", "psz", "psn")):
                    ps = psg.tile([P, 512], F32, tag=tg, name=tg)
                    flat = ps[:, 0 : KO * BATCH]
                    sl = slice(i * KO, (i + 1) * KO)
                    nc.tensor.matmul(
                        flat, i128_sb[:], C_hi[:, sl, :], start=True, stop=False
                    )
                    nc.tensor.matmul(
                        flat, i128_sb[:], C_lo[:, sl, :], start=False, stop=False
                    )
                    tiles.append(flat.rearrange("p (m b) -> p m b", b=BATCH))
                return tiles

            # ---- GRU steps ----
            cur = gate_psums()
            prev = x0_sb
            for t in range(STEPS):
                c, ti = t // TC, t % TC
                for m in range(GM):
                    ps = cur[m // KO]
                    for k in range(KO):
                        nc.tensor.matmul(
                            ps[:, m % KO, :], wih[:, k, m, :], prev[:, k, :],
                            start=False,
                            stop=(k == KO - 1 and m % KO == KO - 1),
                        )
                ps_r, ps_z, ps_n = cur
                nxt = gate_psums()  # preload next step early (no deps on h)

                t_r = sp.tile([P, KO, BATCH], F32, tag="tr")
                nc.scalar.activation(t_r, ps_r[:, :, :], AF.Tanh, scale=0.5)
                t_z = sp.tile([P, KO, BATCH], F32, tag="tz")
                nc.scalar.activation(t_z, ps_z[:, :, :], AF.Tanh, scale=0.5)
                # a = tanh(0.5 s_r) * 0.5*gh_n  (the r-gate modulation)
                a = sp.tile([P, KO, BATCH], F32, tag="a")
                nc.gpsimd.tensor_mul(a, t_r, hn2)
                # sn2 = gi_n + E_n + a   (E_n already in psum)
                sn2 = sp.tile([P, KO, BATCH], F32, tag="sn2")
                nc.vector.tensor_add(sn2, ps_n[:, :, :], a)
                n = sp.tile([P, KO, BATCH], F32, tag="n")
                nc.scalar.activation(n, sn2, AF.Tanh, scale=1.0)
                # h = (0.5 - 0.5 t_z) * n + (h0/2 + 0.5 t_z * h0)
                c1 = sp.tile([P, KO, BATCH], F32, tag="c1")
                nc.gpsimd.scalar_tensor_tensor(
                    c1, t_z, -0.5, halves[:], ALU.mult, ALU.add
                )
                th0 = sp.tile([P, KO, BATCH], F32, tag="th0")
                nc.gpsimd.scalar_tensor_tensor(
                    th0, t_z, 0.5, h0T[:], ALU.mult, ALU.mult
                )
                c2 = sp.tile([P, KO, BATCH], F32, tag="c2")
                nc.gpsimd.tensor_add(c2, th0, h0_half[:])
                m1 = sp.tile([P, KO, BATCH], F32, tag="m1")
                nc.vector.tensor_mul(m1, c1, n)
                dst = res[c][:, :, :, ti]
                nc.vector.tensor_add(dst, m1, c2)
                prev = res[c][:, :, :, ti]
                cur = nxt

                # interleave projection work for the previous chunk
                emit_pending(UNITS_PER_STEP)
                if ti == TC - 1:
                    pending.extend((c, m) for m in range(MT))

            # drain the last chunk's projection
            emit_pending(len(pending))

    nc.compile()
    return nc


def _shard_inputs(feat, W_hp, b_hp, W_ih, W_hh, b_ih, b_hh, embed, W_out, b_out):
    bf = ml_dtypes.bfloat16
    featT = np.ascontiguousarray(feat.T, dtype=np.float32)
    WhpT = np.ascontiguousarray(W_hp.T, dtype=np.float32)
    WihT = np.ascontiguousarray(W_ih.T).astype(bf)
    WhhT = np.ascontiguousarray(W_hh.T, dtype=np.float32)
    x0T = np.ascontiguousarray(
        np.repeat(np.asarray(embed)[SOS][:, None], BATCH, axis=1)
    ).astype(bf)
    Wo = np.zeros((NCORES * VPAD, HID), np.float32)
    Wo[:VOCAB] = W_out
    bo = np.zeros((NCORES * VPAD,), np.float32)
    bo[:VOCAB] = b_out
    common = dict(
        featT=featT, WhpT=WhpT, WihT=WihT, WhhT=WhhT,
        b_ih=np.asarray(b_ih, np.float32), b_hh=np.asarray(b_hh, np.float32),
        b_hp=np.asarray(b_hp, np.float32), x0T=x0T,
        I128=np.eye(P, dtype=np.float32).astype(bf),
    )
    in_maps = []
    for c in range(NCORES):
        sl = slice(c * VPAD, (c + 1) * VPAD)
        m = dict(common)
        m["WoutT"] = np.ascontiguousarray(Wo[sl].T).astype(bf)
        m["b_out"] = bo[sl].copy()
        in_maps.append(m)
    return in_maps


def kernel(**inputs):
    global LAST_RESULTS
    args = {k: np.asarray(v) for k, v in inputs.items()}
    in_maps = _shard_inputs(
        args["feat"], args["W_hp"], args["b_hp"], args["W_ih"], args["W_hh"],
        args["b_ih"], args["b_hh"], args["embed"], args["W_out"], args["b_out"],
    )
    nc = build()
    res = run_bass_kernel_spmd(nc, in_maps, core_ids=list(range(NCORES)))
    LAST_RESULTS = res
    # OUT per core: [MT, P, NCH, B, TC] -> [B, VPAD, T]
    parts = []
    for r in res.results:
        o = r["OUT"]  # [30, 128, 25, 32, 8]
        o = o.transpose(3, 0, 1, 2, 4).reshape(BATCH, VPAD, STEPS)
        parts.append(o)
    out = np.concatenate(parts, axis=1)[:, :VOCAB, :]
    return np.ascontiguousarray(out, dtype=np.float32)


# revision 35
# speedup vs baseline: 2.4252x; 1.1179x over previous
"""Trainium2 Bass kernel for the GRU caption model.

Computes: h0 = feat @ W_hp.T + b_hp; 200-step GRU with constant hidden-proj
gate pre-activations; logits = outs @ W_out.T + b_out -> [B, V, T].

Strategy: every core runs the (tiny, latency-bound) GRU redundantly; the
vocab dimension of W_out is sharded 8 ways; each core emits its own
[B, 3840, T] logits slice which the host concatenates.

Schedule: GRU steps are grouped into 8-step chunks. While the GRU runs
chunk c+1, the vocab projection for chunk c's hidden states is interleaved
into the PE queue a few matmuls per step, keeping the tensor engine
continuously busy (max p-state) and hiding the recurrence's cross-engine
dependency-chain latency. Output DMAs use 1024B contiguous runs; the host
unscrambles the [MT, 128, NCH, B, TC] layout for free.

The constant gate pre-activations (C_rz = gh_rz + b_ih_rz and
E_n = 0.5*gh_n + b_ih_n) are preloaded into PSUM via identity matmuls
(bf16 hi+lo, exact to ~2^-17) so the gate matmuls accumulate on top of
them and tanh reads PSUM directly. The r/z/n gate groups live in three
bank-aligned PSUM tiles so the r-gate tanh fires as soon as the r matmuls
land instead of waiting for the whole gate tile.
"""

import numpy as np
import ml_dtypes

import concourse.bass as bass
import concourse.mybir as mybir
import concourse.tile as tile
from concourse import bacc
from concourse.bass_utils import run_bass_kernel_spmd

F32 = mybir.dt.float32
BF16 = mybir.dt.bfloat16
FP8 = mybir.dt.float8e4
AF = mybir.ActivationFunctionType
ALU = mybir.AluOpType
DR = mybir.MatmulPerfMode.DoubleRow

PSCALE = 8.0           # fp8 pre-scale for W_out and res (epilogue undoes it)

VOCAB = 30522
HID = 512
FEAT = 2048
STEPS = 200
BATCH = 32
SOS = 101
NCORES = 8
P = 128
KO = HID // P          # 4 h-chunks
GM = 3 * HID // P      # 12 gate row-groups (r: 0-3, z: 4-7, n: 8-11)
KF = FEAT // P         # 16 feat chunks
VPAD = 3840            # per-core padded vocab rows = 30 * 128
MT = VPAD // P         # 30 vocab tiles per core
TC = 8                 # GRU steps per projection chunk
NCH = STEPS // TC      # 25 chunks
NPROJ = BATCH * TC     # moving free size per proj matmul = 256

LAST_RESULTS = None    # test harness introspection
UNITS_PER_STEP = 4     # proj units interleaved per GRU step


def build():
    nc = bacc.Bacc("TRN2", target_bir_lowering=False, debug=False)

    featT = nc.dram_tensor("featT", [FEAT, BATCH], F32, kind="ExternalInput")
    WhpT = nc.dram_tensor("WhpT", [FEAT, HID], F32, kind="ExternalInput")
    WihT = nc.dram_tensor("WihT", [HID, 3 * HID], BF16, kind="ExternalInput")
    WhhT = nc.dram_tensor("WhhT", [HID, 3 * HID], F32, kind="ExternalInput")
    b_ih = nc.dram_tensor("b_ih", [3 * HID], F32, kind="ExternalInput")
    b_hh = nc.dram_tensor("b_hh", [3 * HID], F32, kind="ExternalInput")
    b_hp = nc.dram_tensor("b_hp", [HID], F32, kind="ExternalInput")
    x0T = nc.dram_tensor("x0T", [HID, BATCH], BF16, kind="ExternalInput")
    I128 = nc.dram_tensor("I128", [P, P], BF16, kind="ExternalInput")
    Wo8hi = nc.dram_tensor("Wo8hi", [HID, VPAD], FP8, kind="ExternalInput")
    Wo8lo = nc.dram_tensor("Wo8lo", [HID, VPAD], FP8, kind="ExternalInput")
    b_out = nc.dram_tensor("b_out", [VPAD], F32, kind="ExternalInput")
    OUT = nc.dram_tensor("OUT", [MT, P, NCH, BATCH, TC], F32, kind="ExternalOutput")

    with tile.TileContext(nc) as tc:
        with (
            tc.tile_pool(name="const", bufs=1) as const,
            tc.tile_pool(name="stream", bufs=3) as stream,
            tc.tile_pool(name="step", bufs=3) as sp,
            tc.tile_pool(name="hb", bufs=3) as hb,
            tc.tile_pool(name="outp", bufs=6) as outp,
            tc.tile_pool(name="psg", bufs=2, space="PSUM") as psg,
            tc.tile_pool(name="psp", bufs=2, space="PSUM") as psp,
        ):
            # ---- constants into SBUF (order = DMA queue order; the h0/gh
            # weight streams are emitted inside the loops below, and the big
            # wih/wout loads are deferred until after them so they don't
            # block the startup-critical transfers) ----
            featT_sb = const.tile([P, KF, BATCH], F32, tag="featsb")
            nc.sync.dma_start(featT_sb[:], featT.rearrange("(k p) b -> p k b", p=P))
            bih_sb = const.tile([P, GM], F32, tag="bih")
            nc.sync.dma_start(bih_sb[:], b_ih.rearrange("(m p) -> p m", p=P))
            bhh_sb = const.tile([P, GM], F32, tag="bhh")
            nc.sync.dma_start(bhh_sb[:], b_hh.rearrange("(m p) -> p m", p=P))
            bhp_sb = const.tile([P, KO], F32, tag="bhp")
            nc.sync.dma_start(bhp_sb[:], b_hp.rearrange("(m p) -> p m", p=P))
            x0_sb = const.tile([P, KO, BATCH], BF16, tag="x0")
            nc.sync.dma_start(x0_sb[:], x0T.rearrange("(k p) b -> p k b", p=P))
            i128_sb = const.tile([P, P], BF16, tag="i128")
            nc.sync.dma_start(i128_sb[:], I128[:, :])
            halves = const.tile([P, KO, BATCH], F32, tag="halves")
            nc.vector.memset(halves[:], 0.5)

            whp_sb = const.tile([P, KF, HID], F32, tag="whp")
            nc.sync.dma_start(whp_sb[:], WhpT.rearrange("(k p) h -> p k h", p=P))
            whh_sb = const.tile([P, KO, 3 * HID], F32, tag="whh")
            nc.sync.dma_start(whh_sb[:], WhhT.rearrange("(k p) g -> p k g", p=P))

            # ---- h0 = feat @ W_hp.T + b_hp (fp32, exact) ----
            ps_h0 = psg.tile([P, 512], F32, tag="psr", name="psr")
            ps_h = ps_h0[:, 0 : KO * BATCH].rearrange("p (m b) -> p m b", b=BATCH)
            for ko in range(KO):
                for kf in range(KF):
                    nc.tensor.matmul(
                        ps_h[:, ko, :], whp_sb[:, kf, ko * P:(ko + 1) * P],
                        featT_sb[:, kf, :],
                        start=(kf == 0), stop=(kf == KF - 1),
                    )
            h0T = const.tile([P, KO, BATCH], F32, tag="h0T")
            for ko in range(KO):
                nc.scalar.activation(
                    h0T[:, ko, :], ps_h[:, ko, :], AF.Identity,
                    bias=bhp_sb[:, ko, None], scale=1.0,
                )
            h0_half = const.tile([P, KO, BATCH], F32, tag="h0h")
            nc.scalar.mul(h0_half[:], h0T[:], 0.5)

            # ---- gh = h0 @ W_hh.T + b_hh (fp32, exact; step-invariant) ----
            ghT = const.tile([P, GM, BATCH], F32, tag="ghT")
            for part, tg in ((0, "psz"), (1, "psn"), (2, "psr")):
                ps_g0 = psg.tile([P, 512], F32, tag=tg, name=tg)
                ps_g = ps_g0[:, 0 : KO * BATCH].rearrange("p (m b) -> p m b", b=BATCH)
                for mi in range(KO):
                    m = part * KO + mi
                    for k in range(KO):
                        nc.tensor.matmul(
                            ps_g[:, mi, :], whh_sb[:, k, m * P:(m + 1) * P],
                            h0T[:, k, :],
                            start=(k == 0), stop=(k == KO - 1),
                        )
                for mi in range(KO):
                    m = part * KO + mi
                    nc.scalar.activation(
                        ghT[:, m, :], ps_g[:, mi, :], AF.Identity,
                        bias=bhh_sb[:, m, None], scale=1.0,
                    )

            # big resident weights (loaded while h0/gh compute)
            wih = const.tile([P, KO, GM, P], BF16, tag="wih")
            nc.sync.dma_start(
                wih[:], WihT.rearrange("(k p) (m c) -> p k m c", p=P, c=P)
            )
            bout_sb = const.tile([P, MT], F32, tag="bout")
            nc.sync.dma_start(bout_sb[:], b_out.rearrange("(m p) -> p m", p=P))
            wo8hi = const.tile([P, KO, VPAD], FP8, tag="wo8hi")
            nc.sync.dma_start(wo8hi[:], Wo8hi.rearrange("(k p) v -> p k v", p=P))
            wo8lo = const.tile([P, KO, VPAD], FP8, tag="wo8lo")
            nc.sync.dma_start(wo8lo[:], Wo8lo.rearrange("(k p) v -> p k v", p=P))

            # C_all = [C_rz ; E_n]: constant additive gate pre-activations
            C_all = const.tile([P, GM, BATCH], F32, tag="Call")
            nc.vector.tensor_add(
                C_all[:, 0:8, :], ghT[:, 0:8, :],
                bih_sb[:, 0:8, None].to_broadcast((P, 8, BATCH)),
            )
            hn2 = const.tile([P, KO, BATCH], F32, tag="hn2")
            nc.scalar.mul(hn2[:], ghT[:, 8:12, :], 0.5)
            nc.vector.tensor_add(
                C_all[:, 8:12, :], hn2[:],
                bih_sb[:, 8:12, None].to_broadcast((P, KO, BATCH)),
            )
            # split into bf16 hi+lo for exact PE psum preload
            C_hi = const.tile([P, GM, BATCH], BF16, tag="Chi")
            nc.vector.tensor_copy(C_hi[:], C_all[:])
            C_hi32 = const.tile([P, GM, BATCH], F32, tag="Chi32")
            nc.scalar.copy(C_hi32[:], C_hi[:])
            C_lo32 = const.tile([P, GM, BATCH], F32, tag="Clo32")
            nc.vector.tensor_sub(C_lo32[:], C_all[:], C_hi32[:])
            C_lo = const.tile([P, GM, BATCH], BF16, tag="Clo")
            nc.vector.tensor_copy(C_lo[:], C_lo32[:])

            # hidden-state history for the projection: fp8 hi+lo (x PSCALE),
            # chunked by TC steps. (The recurrence itself reads the previous
            # step's h from a small bf16 ring.)
            res8h = [
                const.tile([P, KO, BATCH, TC], FP8, tag=f"r8h{c}", name=f"r8h{c}")
                for c in range(NCH)
            ]
            res8l = [
                const.tile([P, KO, BATCH, TC], FP8, tag=f"r8l{c}", name=f"r8l{c}")
                for c in range(NCH)
            ]

            # ---- projection unit: one vocab tile m of chunk c ----
            # logits*64 = (r8h + r8l) @ Wo8hi + r8h @ Wo8lo  (double-row fp8)
            def proj_unit(c, m):
                ps2 = psp.tile([P, NPROJ], F32, tag="pp", name="pp")
                terms = [(wo8hi, res8h[c]), (wo8hi, res8l[c]), (wo8lo, res8h[c])]
                for i, (w8, r8) in enumerate(terms):
                    for pr in range(KO // 2):
                        nc.tensor.matmul(
                            ps2,
                            w8[:, 2 * pr:2 * pr + 2, m * P:(m + 1) * P],
                            r8[:, 2 * pr:2 * pr + 2, :, :],
                            start=(i == 0 and pr == 0),
                            stop=(i == 2 and pr == KO // 2 - 1),
                            perf_mode=DR,
                        )
                ob = outp.tile([P, NPROJ], F32, tag="ob", name="ob")
                eng = nc.gpsimd if m % 2 == 0 else nc.vector
                eng.scalar_tensor_tensor(
                    ob, ps2, 1.0 / (PSCALE * PSCALE),
                    bout_sb[:, m, None].to_broadcast((P, NPROJ)),
                    ALU.mult, ALU.add,
                )
                nc.sync.dma_start(
                    OUT[m, :, c, :, :], ob.rearrange("p (b t) -> p b t", b=BATCH)
                )

            pending = []     # (c, m) proj units ready to emit
            pend_i = 0

            def emit_pending(limit):
                nonlocal pend_i
                done = 0
                while pend_i < len(pending) and done < limit:
                    proj_unit(*pending[pend_i])
                    pend_i += 1
                    done += 1

            def gate_psums():
                """Allocate r/z/n psum tiles and preload C (start=True)."""
                tiles = []
                for i, tg in enumerate(("psr", "psz", "psn")):
                    ps = psg.tile([P, 512], F32, tag=tg, name=tg)
                    flat = ps[:, 0 : KO * BATCH]
                    sl = slice(i * KO, (i + 1) * KO)
                    nc.tensor.matmul(
                        flat, i128_sb[:], C_hi[:, sl, :], start=True, stop=False
                    )
                    nc.tensor.matmul(
                        flat, i128_sb[:], C_lo[:, sl, :], start=False, stop=False
                    )
                    tiles.append(flat.rearrange("p (m b) -> p m b", b=BATCH))
                return tiles

            # ---- GRU steps ----
            cur = gate_psums()
            prev = x0_sb
            for t in range(STEPS):
                c, ti = t // TC, t % TC
                for m in range(GM):
                    ps = cur[m // KO]
                    for k in range(KO):
                        nc.tensor.matmul(
                            ps[:, m % KO, :], wih[:, k, m, :], prev[:, k, :],
                            start=False,
                            stop=(k == KO - 1 and m % KO == KO - 1),
                        )
                ps_r, ps_z, ps_n = cur
                nxt = gate_psums()  # preload next step early (no deps on h)

                t_r = sp.tile([P, KO, BATCH], F32, tag="tr")
                nc.scalar.activation(t_r, ps_r[:, :, :], AF.Tanh, scale=0.5)
                t_z = sp.tile([P, KO, BATCH], F32, tag="tz")
                nc.scalar.activation(t_z, ps_z[:, :, :], AF.Tanh, scale=0.5)
                # a = tanh(0.5 s_r) * 0.5*gh_n  (the r-gate modulation)
                a = sp.tile([P, KO, BATCH], F32, tag="a")
                nc.gpsimd.tensor_mul(a, t_r, hn2)
                # sn2 = gi_n + E_n + a   (E_n already in psum)
                sn2 = sp.tile([P, KO, BATCH], F32, tag="sn2")
                nc.vector.tensor_add(sn2, ps_n[:, :, :], a)
                n = sp.tile([P, KO, BATCH], F32, tag="n")
                nc.scalar.activation(n, sn2, AF.Tanh, scale=1.0)
                # h = (0.5 - 0.5 t_z) * n + (h0/2 + 0.5 t_z * h0)
                c1 = sp.tile([P, KO, BATCH], F32, tag="c1")
                nc.gpsimd.scalar_tensor_tensor(
                    c1, t_z, -0.5, halves[:], ALU.mult, ALU.add
                )
                th0 = sp.tile([P, KO, BATCH], F32, tag="th0")
                nc.gpsimd.scalar_tensor_tensor(
                    th0, t_z, 0.5, h0T[:], ALU.mult, ALU.mult
                )
                c2 = sp.tile([P, KO, BATCH], F32, tag="c2")
                nc.gpsimd.tensor_add(c2, th0, h0_half[:])
                m1 = sp.tile([P, KO, BATCH], F32, tag="m1")
                nc.vector.tensor_mul(m1, c1, n)
                hq = hb.tile([P, KO, BATCH], BF16, tag="hb")
                nc.vector.tensor_add(hq, m1, c2)
                # fp8 hi+lo copies (x PSCALE) for the projection (off-path)
                r8h_t = res8h[c][:, :, :, ti]
                nc.scalar.activation(r8h_t, hq, AF.Identity, scale=PSCALE)
                hi32 = sp.tile([P, KO, BATCH], F32, tag="hi32")
                nc.vector.tensor_copy(hi32, r8h_t)
                nc.vector.scalar_tensor_tensor(
                    res8l[c][:, :, :, ti], hq, PSCALE, hi32,
                    ALU.mult, ALU.subtract,
                )
                prev = hq
                cur = nxt

                # interleave projection work for the previous chunk
                emit_pending(UNITS_PER_STEP)
                if ti == TC - 1:
                    pending.extend((c, m) for m in range(MT))

            # drain the last chunk's projection
            emit_pending(len(pending))

    nc.compile()
    return nc


def _shard_inputs(feat, W_hp, b_hp, W_ih, W_hh, b_ih, b_hh, embed, W_out, b_out):
    bf = ml_dtypes.bfloat16
    featT = np.ascontiguousarray(feat.T, dtype=np.float32)
    WhpT = np.ascontiguousarray(W_hp.T, dtype=np.float32)
    WihT = np.ascontiguousarray(W_ih.T).astype(bf)
    WhhT = np.ascontiguousarray(W_hh.T, dtype=np.float32)
    x0T = np.ascontiguousarray(
        np.repeat(np.asarray(embed)[SOS][:, None], BATCH, axis=1)
    ).astype(bf)
    Wo = np.zeros((NCORES * VPAD, HID), np.float32)
    Wo[:VOCAB] = W_out
    bo = np.zeros((NCORES * VPAD,), np.float32)
    bo[:VOCAB] = b_out
    f8 = ml_dtypes.float8_e4m3fn
    Wo8hi = (Wo * PSCALE).astype(f8)
    Wo8lo = (Wo * PSCALE - Wo8hi.astype(np.float32)).astype(f8)
    common = dict(
        featT=featT, WhpT=WhpT, WihT=WihT, WhhT=WhhT,
        b_ih=np.asarray(b_ih, np.float32), b_hh=np.asarray(b_hh, np.float32),
        b_hp=np.asarray(b_hp, np.float32), x0T=x0T,
        I128=np.eye(P, dtype=np.float32).astype(bf),
    )
    in_maps = []
    for c in range(NCORES):
        sl = slice(c * VPAD, (c + 1) * VPAD)
        m = dict(common)
        m["Wo8hi"] = np.ascontiguousarray(Wo8hi[sl].T)
        m["Wo8lo"] = np.ascontiguousarray(Wo8lo[sl].T)
        m["b_out"] = bo[sl].copy()
        in_maps.append(m)
    return in_maps


def kernel(**inputs):
    global LAST_RESULTS
    args = {k: np.asarray(v) for k, v in inputs.items()}
    in_maps = _shard_inputs(
        args["feat"], args["W_hp"], args["b_hp"], args["W_ih"], args["W_hh"],
        args["b_ih"], args["b_hh"], args["embed"], args["W_out"], args["b_out"],
    )
    nc = build()
    res = run_bass_kernel_spmd(nc, in_maps, core_ids=list(range(NCORES)))
    LAST_RESULTS = res
    # OUT per core: [MT, P, NCH, B, TC] -> [B, VPAD, T]
    parts = []
    for r in res.results:
        o = r["OUT"]  # [30, 128, 25, 32, 8]
        o = o.transpose(3, 0, 1, 2, 4).reshape(BATCH, VPAD, STEPS)
        parts.append(o)
    out = np.concatenate(parts, axis=1)[:, :VOCAB, :]
    return np.ascontiguousarray(out, dtype=np.float32)
